# revision 3
# baseline (speedup 1.0000x reference)
"""Trainium2 8-core SPMD Bass kernel for a single AttnDecoderRNN step.

Reference computation (per step):
  embedded = emb_table[input_ids]                       (1, B, H)
  attn_w'  = softmax([emb; h] @ attn_w.T + attn_b)      (B, L)
  attn_app = einsum('bl,blh->bh', attn_w', enc)         (B, H)
  x        = relu([emb; attn_app] @ comb_w.T + comb_b)  (B, H)
  GRU step (r, z, n gates)  -> h_new                    (B, H)
  logp     = log_softmax(h_new @ out_w.T + out_b)       (B, V)

Sharding: data-parallel over batch B (front: embedding/attention/GRU),
tensor-parallel over vocab V for the output projection with a sharded
log_softmax (AllGather of h_new, AllReduce of sum-exp).

Layout convention on device: matmuls compute out[M,N] = lhsT[K,M].T @ rhs[K,N].
Activations are kept in [batch, feature] layout; the lhsT (stationary) operand
is the transposed activation [feature, batch], produced either by host-side
pre-transposition (inputs) or on-device PE transposes (intermediates).
"""

import sys

import numpy as np

sys.path.insert(0, "/opt/trn_rl_repo")

from concourse import bass, bacc, mybir, tile  # noqa: E402
from concourse.bass_utils import run_bass_kernel_spmd  # noqa: E402

H, V, L, B = 1024, 32000, 128, 256
NCORES = 8
BS = B // NCORES          # 32 batch rows per core
VS = V // NCORES          # 4000 vocab rows per core
KH = H // 128             # 8 hidden k-chunks
NV = 8                    # vocab n-chunks per core
VC = VS // NV             # 500 columns per vocab chunk
GRU_NC = 3 * H // 512     # 6 n-chunks for the 3H GRU gate matmuls

USE_BF16 = False


def build_graph(use_bf16=USE_BF16):
    f32 = mybir.dt.float32
    i32 = mybir.dt.int32
    dt = mybir.dt.bfloat16 if use_bf16 else f32

    nc = bacc.Bacc("TRN2", target_bir_lowering=False, debug=False,
                   num_devices=NCORES)

    # ---- kernel I/O ----
    ids_ext = nc.dram_tensor("ids", [1, BS], i32, kind="ExternalInput")
    emb_ext = nc.dram_tensor("emb", [V, H], f32, kind="ExternalInput")
    h0T_ext = nc.dram_tensor("h0t", [128, KH * BS], dt, kind="ExternalInput")
    h0_ext = nc.dram_tensor("h0", [BS, H], f32, kind="ExternalInput")
    enc_ext = nc.dram_tensor("enc", [BS, L, H], dt, kind="ExternalInput")
    awT_ext = nc.dram_tensor("attn_wt", [2 * H, L], dt, kind="ExternalInput")
    ab_ext = nc.dram_tensor("attn_b", [1, L], f32, kind="ExternalInput")
    cwT_ext = nc.dram_tensor("comb_wt", [2 * H, H], dt, kind="ExternalInput")
    cb_ext = nc.dram_tensor("comb_b", [1, H], f32, kind="ExternalInput")
    wihT_ext = nc.dram_tensor("w_iht", [H, 3 * H], dt, kind="ExternalInput")
    bih_ext = nc.dram_tensor("b_ih", [1, 3 * H], f32, kind="ExternalInput")
    whhT_ext = nc.dram_tensor("w_hht", [H, 3 * H], dt, kind="ExternalInput")
    bhh_ext = nc.dram_tensor("b_hh", [1, 3 * H], f32, kind="ExternalInput")
    owT_ext = nc.dram_tensor("out_wt", [H, VS], dt, kind="ExternalInput")
    ob_ext = nc.dram_tensor("out_b", [1, VS], f32, kind="ExternalInput")
    ident_ext = nc.dram_tensor("ident", [128, 128], f32, kind="ExternalInput")

    logp_ext = nc.dram_tensor("logp", [B, VS], f32, kind="ExternalOutput")
    hnew_ext = nc.dram_tensor("hnew", [BS, H], f32, kind="ExternalOutput")
    awout_ext = nc.dram_tensor("awout", [BS, L], f32, kind="ExternalOutput")

    with tile.TileContext(nc) as tc:
        from contextlib import ExitStack
        ctx = ExitStack()
        with ctx:
            const = ctx.enter_context(tc.tile_pool(name="const", bufs=1))
            act = ctx.enter_context(tc.tile_pool(name="act", bufs=1))
            encp = ctx.enter_context(tc.tile_pool(name="encp", bufs=3))
            wt = ctx.enter_context(tc.tile_pool(name="wt", bufs=12))
            dram = ctx.enter_context(tc.tile_pool(name="dram", bufs=1,
                                                  space="DRAM"))
            # PSUM pools: 8 banks total.
            pp_attn = ctx.enter_context(
                tc.tile_pool(name="pp_attn", bufs=1, space="PSUM"))
            pp_att = ctx.enter_context(
                tc.tile_pool(name="pp_att", bufs=1, space="PSUM"))
            pp_tr = ctx.enter_context(
                tc.tile_pool(name="pp_tr", bufs=2, space="PSUM"))
            pp_acc = ctx.enter_context(
                tc.tile_pool(name="pp_acc", bufs=3, space="PSUM"))

            # ---- constants / small inputs ----
            ident = const.tile([128, 128], f32)
            nc.sync.dma_start(ident[:], ident_ext[:])
            ones = const.tile([1, 128], f32)
            nc.vector.memset(ones[:], 1.0)
            def bias_tile(ext, lo, n):
                bt = wt.tile([1, n], f32, name="btile", tag="btile", bufs=2)
                nc.sync.dma_start(bt[:], ext[0:1, lo:lo + n])
                return bt
            h0T_sb = const.tile([128, KH * BS], dt)
            nc.sync.dma_start(h0T_sb[:], h0T_ext[:])
            h0_sb = const.tile([BS, H], f32)
            nc.sync.dma_start(h0_sb[:], h0_ext[:])

            def pe_transpose(dst_ap, src_ap, p, m):
                """dst[m,p] (SBUF, any dtype) = src[p,m].T via PE; p=src parts."""
                pt = pp_tr.tile([128, 128], f32)
                nc.tensor.transpose(pt[:m, :p], src_ap, ident[:p, :p])
                nc.vector.tensor_copy(dst_ap, pt[:m, :p])

            # ---- phase 0: embedding gather (data-dependent row DMAs) ----
            ids_sb = const.tile([1, BS], i32)
            emb_rows = act.tile([BS, H], f32)
            gsem = nc.alloc_semaphore("gather_sem")
            with tc.tile_critical():
                nc.gpsimd.dma_start(ids_sb[:], ids_ext[:]).then_inc(gsem, 16)
                nc.gpsimd.wait_ge(gsem, 16)
                for i in range(BS):
                    reg = nc.alloc_register(mybir.EngineType.Pool, f"embidx{i}")
                    nc.gpsimd.load(reg, ids_sb[0:1, i:i + 1])
                    sv = nc.snap(reg, donate=True, min_val=0, max_val=V - 1)
                    nc.gpsimd.dma_start(
                        out=emb_rows[i:i + 1, :],
                        in_=emb_ext[bass.ds(sv, 1), :],
                    ).then_inc(gsem, 16)
                nc.gpsimd.wait_ge(gsem, 16 * (1 + BS))

            # ---- phase 1: embT = emb_rows.T (8 chunks) ----
            embT = act.tile([128, KH * BS], dt)
            for k in range(KH):
                pe_transpose(embT[:, BS * k:BS * (k + 1)],
                             emb_rows[:, 128 * k:128 * (k + 1)], BS, 128)

            # ---- phase 2: attn_logits = [emb, h0] @ attn_w.T + attn_b ----
            pa = pp_attn.tile([BS, L], f32)
            for k in range(2 * KH):
                awt = wt.tile([128, L], dt, tag="wtile")
                nc.sync.dma_start(awt[:], awT_ext[128 * k:128 * (k + 1), :])
                lhsT = (embT[:, BS * k:BS * (k + 1)] if k < KH
                        else h0T_sb[:, BS * (k - KH):BS * (k - KH + 1)])
                nc.tensor.matmul(pa[:], lhsT, awt[:], start=(k == 0),
                                 stop=False)
            ab_sb = bias_tile(ab_ext, 0, L)
            nc.tensor.matmul(pa[:], ones[0:1, :BS], ab_sb[:], start=False,
                             stop=True)

            # ---- phase 3: softmax over L -> aw; awT; block-diag bd ----
            aw_pre = act.tile([BS, L], f32)
            nc.vector.tensor_copy(aw_pre[:], pa[:])
            nmx = act.tile([BS, 1], f32)
            nc.vector.tensor_reduce(nmx[:], aw_pre[:], mybir.AxisListType.X,
                                    mybir.AluOpType.max, negate=True)
            aw_exp = act.tile([BS, L], f32)
            se = act.tile([BS, 1], f32)
            nc.scalar.activation(aw_exp[:], aw_pre[:],
                                 mybir.ActivationFunctionType.Exp,
                                 bias=nmx[:], scale=1.0, accum_out=se[:])
            rse = act.tile([BS, 1], f32)
            nc.vector.reciprocal(rse[:], se[:])
            aw_sb = act.tile([BS, L], f32)
            nc.vector.tensor_scalar_mul(aw_sb[:], aw_exp[:], rse[:])
            nc.sync.dma_start(awout_ext[:], aw_sb[:])

            awT_sb = act.tile([128, BS], dt)
            pe_transpose(awT_sb[:], aw_sb[:], BS, 128)
            # block-diagonal lhsT: bd[:, 32*b + b] = awT[:, b]
            bd = act.tile([128, BS * BS], dt)
            nc.vector.memset(bd[:], 0.0)
            for b in range(BS):
                nc.vector.tensor_copy(bd[:, 33 * b:33 * b + 1],
                                      awT_sb[:, b:b + 1])

            # ---- phase 4: attn_applied[b,:] = aw[b,:] @ enc[b] ----
            patt = pp_att.tile([BS, H], f32)
            for g in range(BS // 2):
                et = encp.tile([128, 2, H], dt, tag="enc")
                nc.sync.dma_start(
                    et[:], enc_ext[2 * g:2 * (g + 1), :, :].transpose([1, 0, 2]))
                for bb in range(2):
                    b = 2 * g + bb
                    for c in range(2):
                        nc.tensor.matmul(
                            patt[:, 512 * c:512 * (c + 1)],
                            bd[:, BS * b:BS * (b + 1)],
                            et[:, bb, 512 * c:512 * (c + 1)],
                            start=(b == 0), stop=(b == BS - 1),
                            skip_group_check=True)
            att_sb = act.tile([BS, H], f32)
            nc.vector.tensor_copy(att_sb[:], patt[:])

            # ---- phase 5: attT ----
            attT = act.tile([128, KH * BS], dt)
            for k in range(KH):
                pe_transpose(attT[:, BS * k:BS * (k + 1)],
                             att_sb[:, 128 * k:128 * (k + 1)], BS, 128)

            # ---- phase 6: x = relu([emb, att] @ comb_w.T + comb_b) ----
            x_sb = act.tile([BS, H], f32)
            for n in range(2):
                px = pp_acc.tile([BS, 512], f32, tag="acc")
                for k in range(2 * KH):
                    cwt = wt.tile([128, 512], dt, tag="wtile")
                    nc.sync.dma_start(
                        cwt[:], cwT_ext[128 * k:128 * (k + 1),
                                        512 * n:512 * (n + 1)])
                    lhsT = (embT[:, BS * k:BS * (k + 1)] if k < KH
                            else attT[:, BS * (k - KH):BS * (k - KH + 1)])
                    nc.tensor.matmul(px[:], lhsT, cwt[:], start=(k == 0),
                                     stop=False)
                cb_sb = bias_tile(cb_ext, 512 * n, 512)
                nc.tensor.matmul(px[:], ones[0:1, :BS], cb_sb[:],
                                 start=False, stop=True)
                nc.scalar.activation(x_sb[:, 512 * n:512 * (n + 1)], px[:],
                                     mybir.ActivationFunctionType.Relu)

            # ---- phase 7: xT ----
            xT = act.tile([128, KH * BS], dt)
            for k in range(KH):
                pe_transpose(xT[:, BS * k:BS * (k + 1)],
                             x_sb[:, 128 * k:128 * (k + 1)], BS, 128)

            # ---- phase 8: GRU gate matmuls gx, gh (B, 3H) ----
            gx_sb = act.tile([BS, 3 * H], f32)
            gh_sb = act.tile([BS, 3 * H], f32)
            for (wext, bext, lT, gout) in (
                    (wihT_ext, bih_ext, xT, gx_sb),
                    (whhT_ext, bhh_ext, h0T_sb, gh_sb)):
                for n in range(GRU_NC):
                    pg = pp_acc.tile([BS, 512], f32, tag="acc")
                    for k in range(KH):
                        gwt = wt.tile([128, 512], dt, tag="wtile")
                        nc.sync.dma_start(
                            gwt[:], wext[128 * k:128 * (k + 1),
                                         512 * n:512 * (n + 1)])
                        nc.tensor.matmul(pg[:], lT[:, BS * k:BS * (k + 1)],
                                         gwt[:], start=(k == 0), stop=False)
                    gb_sb = bias_tile(bext, 512 * n, 512)
                    nc.tensor.matmul(pg[:], ones[0:1, :BS], gb_sb[:],
                                     start=False, stop=True)
                    nc.vector.tensor_copy(gout[:, 512 * n:512 * (n + 1)],
                                          pg[:])

            # ---- phase 9: GRU elementwise -> h_new ----
            t1 = act.tile([BS, H], f32, tag="gtmp", bufs=2)
            r_sb = act.tile([BS, H], f32)
            nc.vector.tensor_add(t1[:], gx_sb[:, 0:H], gh_sb[:, 0:H])
            nc.scalar.activation(r_sb[:], t1[:],
                                 mybir.ActivationFunctionType.Sigmoid)
            t2 = act.tile([BS, H], f32, tag="gtmp", bufs=2)
            z_sb = act.tile([BS, H], f32)
            nc.vector.tensor_add(t2[:], gx_sb[:, H:2 * H], gh_sb[:, H:2 * H])
            nc.scalar.activation(z_sb[:], t2[:],
                                 mybir.ActivationFunctionType.Sigmoid)
            t3 = act.tile([BS, H], f32, tag="gtmp", bufs=2)
            n_sb = act.tile([BS, H], f32)
            nc.vector.tensor_mul(t3[:], r_sb[:], gh_sb[:, 2 * H:3 * H])
            nc.vector.tensor_add(t3[:], t3[:], gx_sb[:, 2 * H:3 * H])
            nc.scalar.activation(n_sb[:], t3[:],
                                 mybir.ActivationFunctionType.Tanh)
            t4 = act.tile([BS, H], f32, tag="gtmp", bufs=2)
            hnew_sb = act.tile([BS, H], f32)
            nc.vector.tensor_sub(t4[:], h0_sb[:], n_sb[:])
            nc.vector.tensor_mul(t4[:], z_sb[:], t4[:])
            nc.vector.tensor_add(hnew_sb[:], n_sb[:], t4[:])
            nc.sync.dma_start(hnew_ext[:], hnew_sb[:])

            # ---- phase 9b: AllGather h_new across cores ----
            hnew_bounce = dram.tile([BS, H], f32)
            h_all_dram = dram.tile([B, H], f32, addr_space="Shared")
            nc.sync.dma_start(hnew_bounce[:], hnew_sb[:])
            nc.gpsimd.collective_compute(
                "AllGather", mybir.AluOpType.bypass,
                replica_groups=[list(range(NCORES))],
                ins=[hnew_bounce[:]], outs=[h_all_dram[:]])
            h_allT = []
            for m in range(2):
                ha = act.tile([128, H], f32, tag="h_all")
                nc.sync.dma_start(ha[:], h_all_dram[128 * m:128 * (m + 1), :])
                for k in range(KH):
                    if m == 0:
                        h_allT.append(act.tile([128, 2 * 128], dt,
                                               name=f"h_allT{k}",
                                               tag=f"h_allT{k}"))
                    pe_transpose(h_allT[k][:, 128 * m:128 * (m + 1)],
                                 ha[:, 128 * k:128 * (k + 1)], 128, 128)

            # ---- phase 10/11: logits = h_all @ out_w.T + b; exp-sums ----
            logits = []
            lsums = act.tile([128, 2 * NV], f32)
            for m in range(2):
                lg = act.tile([128, VS], f32, tag=f"logits{m}")
                logits.append(lg)
                for n in range(NV):
                    pl = pp_acc.tile([128, VC], f32, tag="acc")
                    for k in range(KH):
                        owt = wt.tile([128, VC], dt, tag="wtile")
                        nc.sync.dma_start(
                            owt[:], owT_ext[128 * k:128 * (k + 1),
                                            VC * n:VC * (n + 1)])
                        nc.tensor.matmul(
                            pl[:], h_allT[k][:, 128 * m:128 * (m + 1)],
                            owt[:], start=(k == 0), stop=False)
                    ob_sb = bias_tile(ob_ext, VC * n, VC)
                    nc.tensor.matmul(pl[:], ones[:], ob_sb[:],
                                     start=False, stop=True)
                    nc.vector.tensor_copy(lg[:, VC * n:VC * (n + 1)], pl[:])
                    esc = act.tile([128, VC], f32, tag="escratch")
                    nc.scalar.activation(
                        esc[:], lg[:, VC * n:VC * (n + 1)],
                        mybir.ActivationFunctionType.Exp,
                        accum_out=lsums[:, NV * m + n:NV * m + n + 1])

            # ---- phase 12: global log-sum-exp via AllReduce ----
            lsum_sb = act.tile([128, 2], f32)
            for m in range(2):
                nc.vector.tensor_reduce(lsum_sb[:, m:m + 1],
                                        lsums[:, NV * m:NV * (m + 1)],
                                        mybir.AxisListType.X,
                                        mybir.AluOpType.add)
            lsumT_sb = act.tile([2, 128], f32)
            pe_transpose(lsumT_sb[:], lsum_sb[:], 128, 2)
            lsumT_dram = dram.tile([2, 128], f32)
            gsumT_dram = dram.tile([2, 128], f32, addr_space="Shared")
            nc.sync.dma_start(lsumT_dram[:], lsumT_sb[:])
            nc.gpsimd.collective_compute(
                "AllReduce", mybir.AluOpType.add,
                replica_groups=[list(range(NCORES))],
                ins=[lsumT_dram[:]], outs=[gsumT_dram[:]])
            gsumT_sb = act.tile([2, 128], f32)
            nc.sync.dma_start(gsumT_sb[:], gsumT_dram[:])
            loggT_sb = act.tile([2, 128], f32)
            nc.scalar.activation(loggT_sb[:], gsumT_sb[:],
                                 mybir.ActivationFunctionType.Ln)
            logg_sb = act.tile([128, 2], f32)
            pe_transpose(logg_sb[:], loggT_sb[:], 2, 128)

            # ---- phase 13: logp = logits - log(gsum); write out ----
            for m in range(2):
                nc.vector.tensor_scalar_sub(logits[m][:], logits[m][:],
                                            logg_sb[:, m:m + 1])
                nc.sync.dma_start(logp_ext[128 * m:128 * (m + 1), :],
                                  logits[m][:])

    nc.compile()
    return nc


def stage_inputs(input_ids, hidden, encoder_outputs, emb_table,
                 attn_w, attn_b, comb_w, comb_b,
                 w_ih, b_ih, w_hh, b_hh, out_w, out_b, use_bf16=USE_BF16):
    if use_bf16:
        import ml_dtypes
        np_dt = ml_dtypes.bfloat16
    else:
        np_dt = np.float32
    f32 = np.float32

    def cvt(x, dtype):
        return np.ascontiguousarray(np.asarray(x), dtype=dtype)

    ids = np.asarray(input_ids).astype(np.int32).reshape(1, B)
    h0 = cvt(hidden, f32).reshape(B, H)
    enc = cvt(encoder_outputs, np_dt)
    emb = cvt(emb_table, f32)
    awT = cvt(np.asarray(attn_w).T, np_dt)
    ab = cvt(attn_b, f32).reshape(1, L)
    cwT = cvt(np.asarray(comb_w).T, np_dt)
    cb = cvt(comb_b, f32).reshape(1, H)
    wihT = cvt(np.asarray(w_ih).T, np_dt)
    bih = cvt(b_ih, f32).reshape(1, 3 * H)
    whhT = cvt(np.asarray(w_hh).T, np_dt)
    bhh = cvt(b_hh, f32).reshape(1, 3 * H)
    owT_full = np.asarray(out_w).T  # (H, V)
    ob = cvt(out_b, f32).reshape(1, V)
    ident = np.eye(128, dtype=f32)

    in_maps = []
    for j in range(NCORES):
        bsl = slice(BS * j, BS * (j + 1))
        vsl = slice(VS * j, VS * (j + 1))
        h0_j = np.ascontiguousarray(h0[bsl])
        # packed transposed hidden: h0T[p, BS*k + b] = h0_j[b, 128k + p]
        h0T_j = np.ascontiguousarray(
            h0_j.T.reshape(KH, 128, BS).transpose(1, 0, 2).reshape(128, KH * BS),
            dtype=np_dt)
        in_maps.append({
            "ids": np.ascontiguousarray(ids[:, bsl]),
            "emb": emb,
            "h0t": h0T_j,
            "h0": h0_j,
            "enc": np.ascontiguousarray(enc[bsl]),
            "attn_wt": awT,
            "attn_b": ab,
            "comb_wt": cwT,
            "comb_b": cb,
            "w_iht": wihT,
            "b_ih": bih,
            "w_hht": whhT,
            "b_hh": bhh,
            "out_wt": np.ascontiguousarray(owT_full[:, vsl], dtype=np_dt),
            "out_b": np.ascontiguousarray(ob[:, vsl]),
            "ident": ident,
        })
    return in_maps


def run(inputs, trace=False, trace_cores=None, use_bf16=USE_BF16):
    nc = build_graph(use_bf16)
    in_maps = stage_inputs(**inputs, use_bf16=use_bf16)
    res = run_bass_kernel_spmd(
        nc, in_maps, core_ids=list(range(NCORES)),
        trace=trace, trace_cores=trace_cores)
    r = res.results
    logp = np.concatenate([r[j]["logp"] for j in range(NCORES)], axis=1)
    hnew = np.concatenate([r[j]["hnew"] for j in range(NCORES)], axis=0)[None]
    aw = np.concatenate([r[j]["awout"] for j in range(NCORES)], axis=0)
    return (logp, hnew, aw), res


def kernel(**inputs):
    outs, _ = run(inputs, trace=False)
    return outs


# revision 5
# speedup vs baseline: 1.2606x; 1.2606x over previous
"""Trainium2 8-core SPMD Bass kernel for a single AttnDecoderRNN step.

Reference computation (per step):
  embedded = emb_table[input_ids]                       (1, B, H)
  attn_w'  = softmax([emb; h] @ attn_w.T + attn_b)      (B, L)
  attn_app = einsum('bl,blh->bh', attn_w', enc)         (B, H)
  x        = relu([emb; attn_app] @ comb_w.T + comb_b)  (B, H)
  GRU step (r, z, n gates)  -> h_new                    (B, H)
  logp     = log_softmax(h_new @ out_w.T + out_b)       (B, V)

Sharding: data-parallel over batch B (front: embedding/attention/GRU),
tensor-parallel over vocab V for the output projection with a sharded
log_softmax (AllGather of h_new, AllReduce of sum-exp).

Layout convention on device: matmuls compute out[M,N] = lhsT[K,M].T @ rhs[K,N].
Activations are kept in [batch, feature] layout; the lhsT (stationary) operand
is the transposed activation [feature, batch], produced either by host-side
pre-transposition (inputs) or on-device PE transposes (intermediates).
Weight streams use wide per-k-chunk tiles (one DMA per 128-row chunk) with
k-outer / n-inner matmul loops accumulating into per-n PSUM banks.
"""

import sys

import numpy as np

sys.path.insert(0, "/opt/trn_rl_repo")

from concourse import bass, bacc, mybir, tile  # noqa: E402
from concourse.bass_utils import run_bass_kernel_spmd  # noqa: E402

H, V, L, B = 1024, 32000, 128, 256
NCORES = 8
BS = B // NCORES          # 32 batch rows per core
VS = V // NCORES          # 4000 vocab rows per core
KH = H // 128             # 8 hidden k-chunks
NV = 8                    # vocab n-chunks per core
VC = VS // NV             # 500 columns per vocab chunk
GRU_NC = 3 * H // 512     # 6 n-chunks for the 3H GRU gate matmuls
EB = 4                    # encoder batch rows per DMA tile

USE_BF16 = True


def build_graph(use_bf16=USE_BF16):
    f32 = mybir.dt.float32
    i32 = mybir.dt.int32
    dt = mybir.dt.bfloat16 if use_bf16 else f32

    nc = bacc.Bacc("TRN2", target_bir_lowering=False, debug=False,
                   num_devices=NCORES)

    # ---- kernel I/O ----
    ids_ext = nc.dram_tensor("ids", [1, BS], i32, kind="ExternalInput")
    emb_ext = nc.dram_tensor("emb", [V, H], f32, kind="ExternalInput")
    h0T_ext = nc.dram_tensor("h0t", [128, KH * BS], dt, kind="ExternalInput")
    h0_ext = nc.dram_tensor("h0", [BS, H], f32, kind="ExternalInput")
    # encoder outputs packed as enc[l, B_s*1024 + h] = enc_orig[b, l, h]
    enc_ext = nc.dram_tensor("enc", [L, BS * H], dt, kind="ExternalInput")
    # attn_w.T packed as awT[p, 128*k + l] = attn_w[l, 128*k + p]
    awT_ext = nc.dram_tensor("attn_wt", [128, 2 * KH * L], dt,
                             kind="ExternalInput")
    ab_ext = nc.dram_tensor("attn_b", [1, L], f32, kind="ExternalInput")
    cwT_ext = nc.dram_tensor("comb_wt", [2 * H, H], dt, kind="ExternalInput")
    cb_ext = nc.dram_tensor("comb_b", [1, H], f32, kind="ExternalInput")
    wihT_ext = nc.dram_tensor("w_iht", [H, 3 * H], dt, kind="ExternalInput")
    bih_ext = nc.dram_tensor("b_ih", [1, 3 * H], f32, kind="ExternalInput")
    whhT_ext = nc.dram_tensor("w_hht", [H, 3 * H], dt, kind="ExternalInput")
    bhh_ext = nc.dram_tensor("b_hh", [1, 3 * H], f32, kind="ExternalInput")
    owT_ext = nc.dram_tensor("out_wt", [H, VS], dt, kind="ExternalInput")
    ob_ext = nc.dram_tensor("out_b", [1, VS], f32, kind="ExternalInput")
    ident_ext = nc.dram_tensor("ident", [128, 128], f32, kind="ExternalInput")

    logp_ext = nc.dram_tensor("logp", [B, VS], f32, kind="ExternalOutput")
    hnew_ext = nc.dram_tensor("hnew", [BS, H], f32, kind="ExternalOutput")
    awout_ext = nc.dram_tensor("awout", [BS, L], f32, kind="ExternalOutput")

    with tile.TileContext(nc) as tc:
        from contextlib import ExitStack
        ctx = ExitStack()
        with ctx:
            const = ctx.enter_context(tc.tile_pool(name="const", bufs=1))
            act = ctx.enter_context(tc.tile_pool(name="act", bufs=1))
            wt = ctx.enter_context(tc.tile_pool(name="wt", bufs=2))
            dram = ctx.enter_context(tc.tile_pool(name="dram", bufs=1,
                                                  space="DRAM"))
            pp = ctx.enter_context(tc.tile_pool(name="pp", bufs=8,
                                                space="PSUM"))

            def psum(name):
                return pp.tile([BS, 512], f32, name=name, tag="ps")

            def psum128(name):
                return pp.tile([128, 512], f32, name=name, tag="ps")

            # ---- constants / small inputs ----
            ident = const.tile([128, 128], f32)
            nc.sync.dma_start(ident[:], ident_ext[:])
            ones = const.tile([1, 128], f32)
            nc.vector.memset(ones[:], 1.0)

            def bias_tile(ext, lo, n):
                bt = wt.tile([1, n], f32, name="btile", tag="btile", bufs=2)
                nc.sync.dma_start(bt[:], ext[0:1, lo:lo + n])
                return bt

            h0T_sb = const.tile([128, KH * BS], dt)
            nc.sync.dma_start(h0T_sb[:], h0T_ext[:])
            h0_sb = const.tile([BS, H], f32)
            nc.sync.dma_start(h0_sb[:], h0_ext[:])

            def pe_transpose(dst_ap, src_ap, p, m):
                """dst[m,p] (SBUF, any dtype) = src[p,m].T via PE; p=src parts."""
                pt = psum128("pt")
                nc.tensor.transpose(pt[:m, :p], src_ap, ident[:p, :p])
                nc.vector.tensor_copy(dst_ap, pt[:m, :p])

            # ---- embedding gather (data-dependent row DMAs, gpsimd) ----
            ids_sb = const.tile([1, BS], i32)
            emb_rows = act.tile([BS, H], f32)
            gsem = nc.alloc_semaphore("gather_sem")
            with tc.tile_critical():
                nc.gpsimd.dma_start(ids_sb[:], ids_ext[:]).then_inc(gsem, 16)
                nc.gpsimd.wait_ge(gsem, 16)
                for i in range(BS):
                    reg = nc.alloc_register(mybir.EngineType.Pool, f"embidx{i}")
                    nc.gpsimd.load(reg, ids_sb[0:1, i:i + 1])
                    sv = nc.snap(reg, donate=True, min_val=0, max_val=V - 1)
                    nc.gpsimd.dma_start(
                        out=emb_rows[i:i + 1, :],
                        in_=emb_ext[bass.ds(sv, 1), :],
                    ).then_inc(gsem, 16)
                nc.gpsimd.wait_ge(gsem, 16 * (1 + BS))

            # ---- GRU gh = h0 @ w_hh.T + b_hh (independent of the gather,
            #      runs first so PE has dense work while the gather runs) ----
            gh_sb = act.tile([BS, 3 * H], f32)
            pgs = [psum(f"pgh{n}") for n in range(GRU_NC)]
            for k in range(KH):
                gwt = wt.tile([128, 3 * H], dt, name="gwt", tag="w3072",
                              bufs=2)
                nc.sync.dma_start(gwt[:], whhT_ext[128 * k:128 * (k + 1), :])
                for n in range(GRU_NC):
                    nc.tensor.matmul(pgs[n][:], h0T_sb[:, BS * k:BS * (k + 1)],
                                     gwt[:, 512 * n:512 * (n + 1)],
                                     start=(k == 0), stop=False,
                                     skip_group_check=True)
            for n in range(GRU_NC):
                gb = bias_tile(bhh_ext, 512 * n, 512)
                nc.tensor.matmul(pgs[n][:], ones[0:1, :BS], gb[:],
                                 start=False, stop=True, skip_group_check=True)
                nc.vector.tensor_copy(gh_sb[:, 512 * n:512 * (n + 1)],
                                      pgs[n][:])

            # ---- embT ----
            embT = act.tile([128, KH * BS], dt)
            for k in range(KH):
                pe_transpose(embT[:, BS * k:BS * (k + 1)],
                             emb_rows[:, 128 * k:128 * (k + 1)], BS, 128)

            # ---- attn_logits = [emb, h0] @ attn_w.T + attn_b ----
            awt = wt.tile([128, 2 * KH * L], dt, name="awt", tag="wattn",
                          bufs=1)
            nc.sync.dma_start(awt[:], awT_ext[:])
            pa = psum("pa")
            for k in range(2 * KH):
                lhsT = (embT[:, BS * k:BS * (k + 1)] if k < KH
                        else h0T_sb[:, BS * (k - KH):BS * (k - KH + 1)])
                nc.tensor.matmul(pa[:, :L], lhsT,
                                 awt[:, L * k:L * (k + 1)],
                                 start=(k == 0), stop=False)
            ab_sb = bias_tile(ab_ext, 0, L)
            nc.tensor.matmul(pa[:, :L], ones[0:1, :BS], ab_sb[:], start=False,
                             stop=True)

            # ---- softmax over L -> aw; awT; block-diag bd ----
            aw_pre = act.tile([BS, L], f32)
            nc.vector.tensor_copy(aw_pre[:], pa[:, :L])
            nmx = act.tile([BS, 1], f32)
            nc.vector.tensor_reduce(nmx[:], aw_pre[:], mybir.AxisListType.X,
                                    mybir.AluOpType.max, negate=True)
            aw_exp = act.tile([BS, L], f32)
            se = act.tile([BS, 1], f32)
            nc.scalar.activation(aw_exp[:], aw_pre[:],
                                 mybir.ActivationFunctionType.Exp,
                                 bias=nmx[:], scale=1.0, accum_out=se[:])
            rse = act.tile([BS, 1], f32)
            nc.vector.reciprocal(rse[:], se[:])
            aw_sb = act.tile([BS, L], f32)
            nc.vector.tensor_scalar_mul(aw_sb[:], aw_exp[:], rse[:])
            nc.sync.dma_start(awout_ext[:], aw_sb[:])

            awT_sb = act.tile([128, BS], dt)
            pe_transpose(awT_sb[:], aw_sb[:], BS, 128)
            # block-diagonal lhsT: bd[:, 32*b + b] = awT[:, b]
            bd = act.tile([128, BS * BS], dt)
            nc.vector.memset(bd[:], 0.0)
            for b in range(BS):
                nc.vector.tensor_copy(bd[:, 33 * b:33 * b + 1],
                                      awT_sb[:, b:b + 1])

            # ---- attn_applied[b,:] = aw[b,:] @ enc[b] ----
            patt = [psum("patt0"), psum("patt1")]
            for g in range(BS // EB):
                et = wt.tile([128, EB * H], dt, name="enc", tag="enc", bufs=2)
                nc.sync.dma_start(et[:], enc_ext[:, EB * H * g:
                                                 EB * H * (g + 1)])
                for bb in range(EB):
                    b = EB * g + bb
                    for c in range(2):
                        nc.tensor.matmul(
                            patt[c][:],
                            bd[:, BS * b:BS * (b + 1)],
                            et[:, H * bb + 512 * c:H * bb + 512 * (c + 1)],
                            start=(b == 0), stop=(b == BS - 1),
                            skip_group_check=True)
            att_sb = act.tile([BS, H], f32)
            for c in range(2):
                nc.vector.tensor_copy(att_sb[:, 512 * c:512 * (c + 1)],
                                      patt[c][:])

            # ---- attT ----
            attT = act.tile([128, KH * BS], dt)
            for k in range(KH):
                pe_transpose(attT[:, BS * k:BS * (k + 1)],
                             att_sb[:, 128 * k:128 * (k + 1)], BS, 128)

            # ---- x = relu([emb, att] @ comb_w.T + comb_b) ----
            x_sb = act.tile([BS, H], f32)
            pxs = [psum("px0"), psum("px1")]
            for k in range(2 * KH):
                cwt = wt.tile([128, H], dt, name="cwt", tag="w1024", bufs=2)
                nc.sync.dma_start(cwt[:], cwT_ext[128 * k:128 * (k + 1), :])
                lhsT = (embT[:, BS * k:BS * (k + 1)] if k < KH
                        else attT[:, BS * (k - KH):BS * (k - KH + 1)])
                for n in range(2):
                    nc.tensor.matmul(pxs[n][:], lhsT,
                                     cwt[:, 512 * n:512 * (n + 1)],
                                     start=(k == 0), stop=False,
                                     skip_group_check=True)
            for n in range(2):
                cb_sb = bias_tile(cb_ext, 512 * n, 512)
                nc.tensor.matmul(pxs[n][:], ones[0:1, :BS], cb_sb[:],
                                 start=False, stop=True, skip_group_check=True)
                nc.scalar.activation(x_sb[:, 512 * n:512 * (n + 1)],
                                     pxs[n][:],
                                     mybir.ActivationFunctionType.Relu)

            # ---- xT ----
            xT = act.tile([128, KH * BS], dt)
            for k in range(KH):
                pe_transpose(xT[:, BS * k:BS * (k + 1)],
                             x_sb[:, 128 * k:128 * (k + 1)], BS, 128)

            # ---- gx = x @ w_ih.T + b_ih ----
            gx_sb = act.tile([BS, 3 * H], f32)
            pgx = [psum(f"pgx{n}") for n in range(GRU_NC)]
            for k in range(KH):
                gwt2 = wt.tile([128, 3 * H], dt, name="gwt2", tag="w3072",
                               bufs=2)
                nc.sync.dma_start(gwt2[:], wihT_ext[128 * k:128 * (k + 1), :])
                for n in range(GRU_NC):
                    nc.tensor.matmul(pgx[n][:], xT[:, BS * k:BS * (k + 1)],
                                     gwt2[:, 512 * n:512 * (n + 1)],
                                     start=(k == 0), stop=False,
                                     skip_group_check=True)
            for n in range(GRU_NC):
                gb2 = bias_tile(bih_ext, 512 * n, 512)
                nc.tensor.matmul(pgx[n][:], ones[0:1, :BS], gb2[:],
                                 start=False, stop=True, skip_group_check=True)
                nc.vector.tensor_copy(gx_sb[:, 512 * n:512 * (n + 1)],
                                      pgx[n][:])

            # ---- GRU elementwise -> h_new ----
            t1 = act.tile([BS, H], f32, tag="gtmp", bufs=2)
            r_sb = act.tile([BS, H], f32)
            nc.vector.tensor_add(t1[:], gx_sb[:, 0:H], gh_sb[:, 0:H])
            nc.scalar.activation(r_sb[:], t1[:],
                                 mybir.ActivationFunctionType.Sigmoid)
            t2 = act.tile([BS, H], f32, tag="gtmp", bufs=2)
            z_sb = act.tile([BS, H], f32)
            nc.vector.tensor_add(t2[:], gx_sb[:, H:2 * H], gh_sb[:, H:2 * H])
            nc.scalar.activation(z_sb[:], t2[:],
                                 mybir.ActivationFunctionType.Sigmoid)
            t3 = act.tile([BS, H], f32, tag="gtmp", bufs=2)
            n_sb = act.tile([BS, H], f32)
            nc.vector.tensor_mul(t3[:], r_sb[:], gh_sb[:, 2 * H:3 * H])
            nc.vector.tensor_add(t3[:], t3[:], gx_sb[:, 2 * H:3 * H])
            nc.scalar.activation(n_sb[:], t3[:],
                                 mybir.ActivationFunctionType.Tanh)
            t4 = act.tile([BS, H], f32, tag="gtmp", bufs=2)
            hnew_sb = act.tile([BS, H], f32)
            nc.vector.tensor_sub(t4[:], h0_sb[:], n_sb[:])
            nc.vector.tensor_mul(t4[:], z_sb[:], t4[:])
            nc.vector.tensor_add(hnew_sb[:], n_sb[:], t4[:])
            nc.sync.dma_start(hnew_ext[:], hnew_sb[:])

            # ---- AllGather h_new across cores ----
            hnew_bounce = dram.tile([BS, H], f32)
            h_all_dram = dram.tile([B, H], f32, addr_space="Shared")
            nc.sync.dma_start(hnew_bounce[:], hnew_sb[:])
            nc.gpsimd.collective_compute(
                "AllGather", mybir.AluOpType.bypass,
                replica_groups=[list(range(NCORES))],
                ins=[hnew_bounce[:]], outs=[h_all_dram[:]])
            h_allT = []
            for m in range(2):
                ha = act.tile([128, H], f32, tag="h_all", name=f"h_all{m}")
                nc.sync.dma_start(ha[:], h_all_dram[128 * m:128 * (m + 1), :])
                for k in range(KH):
                    if m == 0:
                        h_allT.append(act.tile([128, 2 * 128], dt,
                                               name=f"h_allT{k}",
                                               tag=f"h_allT{k}"))
                    pe_transpose(h_allT[k][:, 128 * m:128 * (m + 1)],
                                 ha[:, 128 * k:128 * (k + 1)], 128, 128)

            # ---- logits = h_all @ out_w.T + out_b; exp partial sums ----
            logits = []
            lsums = act.tile([128, 2 * NV], f32)
            for m in range(2):
                lg = act.tile([128, VS], f32, tag=f"logits{m}",
                              name=f"logits{m}")
                logits.append(lg)
                pls = [pp.tile([128, VC], f32, name=f"pl{n}", tag="ps")
                       for n in range(NV)]
                for k in range(KH):
                    owt = wt.tile([128, VS], dt, name="owt", tag="w4000",
                                  bufs=2)
                    nc.sync.dma_start(owt[:],
                                      owT_ext[128 * k:128 * (k + 1), :])
                    for n in range(NV):
                        nc.tensor.matmul(
                            pls[n][:], h_allT[k][:, 128 * m:128 * (m + 1)],
                            owt[:, VC * n:VC * (n + 1)],
                            start=(k == 0), stop=False,
                            skip_group_check=True)
                for n in range(NV):
                    ob_sb = bias_tile(ob_ext, VC * n, VC)
                    nc.tensor.matmul(pls[n][:], ones[:], ob_sb[:],
                                     start=False, stop=True,
                                     skip_group_check=True)
                    nc.vector.tensor_copy(lg[:, VC * n:VC * (n + 1)],
                                          pls[n][:])
                    esc = act.tile([128, VC], f32, tag="escratch",
                                   name="esc")
                    nc.scalar.activation(
                        esc[:], lg[:, VC * n:VC * (n + 1)],
                        mybir.ActivationFunctionType.Exp,
                        accum_out=lsums[:, NV * m + n:NV * m + n + 1])

            # ---- global log-sum-exp via AllReduce ----
            lsum_sb = act.tile([128, 2], f32)
            for m in range(2):
                nc.vector.tensor_reduce(lsum_sb[:, m:m + 1],
                                        lsums[:, NV * m:NV * (m + 1)],
                                        mybir.AxisListType.X,
                                        mybir.AluOpType.add)
            lsumT_sb = act.tile([2, 128], f32)
            pe_transpose(lsumT_sb[:], lsum_sb[:], 128, 2)
            lsumT_dram = dram.tile([2, 128], f32)
            gsumT_dram = dram.tile([2, 128], f32, addr_space="Shared")
            nc.sync.dma_start(lsumT_dram[:], lsumT_sb[:])
            nc.gpsimd.collective_compute(
                "AllReduce", mybir.AluOpType.add,
                replica_groups=[list(range(NCORES))],
                ins=[lsumT_dram[:]], outs=[gsumT_dram[:]])
            gsumT_sb = act.tile([2, 128], f32)
            nc.sync.dma_start(gsumT_sb[:], gsumT_dram[:])
            loggT_sb = act.tile([2, 128], f32)
            nc.scalar.activation(loggT_sb[:], gsumT_sb[:],
                                 mybir.ActivationFunctionType.Ln)
            logg_sb = act.tile([128, 2], f32)
            pe_transpose(logg_sb[:], loggT_sb[:], 2, 128)

            # ---- logp = logits - log(gsum); write out ----
            for m in range(2):
                nc.vector.tensor_scalar_sub(logits[m][:], logits[m][:],
                                            logg_sb[:, m:m + 1])
                nc.sync.dma_start(logp_ext[128 * m:128 * (m + 1), :],
                                  logits[m][:])

    nc.compile()
    return nc


def stage_inputs(input_ids, hidden, encoder_outputs, emb_table,
                 attn_w, attn_b, comb_w, comb_b,
                 w_ih, b_ih, w_hh, b_hh, out_w, out_b, use_bf16=USE_BF16):
    if use_bf16:
        import ml_dtypes
        np_dt = ml_dtypes.bfloat16
    else:
        np_dt = np.float32
    f32 = np.float32

    def cvt(x, dtype):
        return np.ascontiguousarray(np.asarray(x), dtype=dtype)

    ids = np.asarray(input_ids).astype(np.int32).reshape(1, B)
    h0 = cvt(hidden, f32).reshape(B, H)
    enc = cvt(encoder_outputs, np_dt)
    emb = cvt(emb_table, f32)
    # attn_w.T (2H, L) packed into (128, 16*L): [p, L*k + l] = attn_w.T[128k+p, l]
    awT = np.asarray(attn_w).T.astype(np_dt)  # (2H, L)
    awT_packed = np.ascontiguousarray(
        awT.reshape(2 * KH, 128, L).transpose(1, 0, 2).reshape(128, 2 * KH * L))
    ab = cvt(attn_b, f32).reshape(1, L)
    cwT = cvt(np.asarray(comb_w).T, np_dt)
    cb = cvt(comb_b, f32).reshape(1, H)
    wihT = cvt(np.asarray(w_ih).T, np_dt)
    bih = cvt(b_ih, f32).reshape(1, 3 * H)
    whhT = cvt(np.asarray(w_hh).T, np_dt)
    bhh = cvt(b_hh, f32).reshape(1, 3 * H)
    owT_full = np.asarray(out_w).T  # (H, V)
    ob = cvt(out_b, f32).reshape(1, V)
    ident = np.eye(128, dtype=f32)

    in_maps = []
    for j in range(NCORES):
        bsl = slice(BS * j, BS * (j + 1))
        vsl = slice(VS * j, VS * (j + 1))
        h0_j = np.ascontiguousarray(h0[bsl])
        # packed transposed hidden: h0T[p, BS*k + b] = h0_j[b, 128k + p]
        h0T_j = np.ascontiguousarray(
            h0_j.T.reshape(KH, 128, BS).transpose(1, 0, 2).reshape(128, KH * BS),
            dtype=np_dt)
        # encoder shard packed as (L, BS*H): [l, 1024*b + h]
        enc_j = np.ascontiguousarray(
            enc[bsl].transpose(1, 0, 2).reshape(L, BS * H))
        in_maps.append({
            "ids": np.ascontiguousarray(ids[:, bsl]),
            "emb": emb,
            "h0t": h0T_j,
            "h0": h0_j,
            "enc": enc_j,
            "attn_wt": awT_packed,
            "attn_b": ab,
            "comb_wt": cwT,
            "comb_b": cb,
            "w_iht": wihT,
            "b_ih": bih,
            "w_hht": whhT,
            "b_hh": bhh,
            "out_wt": np.ascontiguousarray(owT_full[:, vsl], dtype=np_dt),
            "out_b": np.ascontiguousarray(ob[:, vsl]),
            "ident": ident,
        })
    return in_maps


def run(inputs, trace=False, trace_cores=None, use_bf16=USE_BF16):
    nc = build_graph(use_bf16)
    in_maps = stage_inputs(**inputs, use_bf16=use_bf16)
    res = run_bass_kernel_spmd(
        nc, in_maps, core_ids=list(range(NCORES)),
        trace=trace, trace_cores=trace_cores)
    r = res.results
    logp = np.concatenate([r[j]["logp"] for j in range(NCORES)], axis=1)
    hnew = np.concatenate([r[j]["hnew"] for j in range(NCORES)], axis=0)[None]
    aw = np.concatenate([r[j]["awout"] for j in range(NCORES)], axis=0)
    return (logp, hnew, aw), res


def kernel(**inputs):
    outs, _ = run(inputs, trace=False)
    return outs


# revision 7
# speedup vs baseline: 1.3979x; 1.1089x over previous
"""Trainium2 8-core SPMD Bass kernel for a single AttnDecoderRNN step.

Reference computation (per step):
  embedded = emb_table[input_ids]                       (1, B, H)
  attn_w'  = softmax([emb; h] @ attn_w.T + attn_b)      (B, L)
  attn_app = einsum('bl,blh->bh', attn_w', enc)         (B, H)
  x        = relu([emb; attn_app] @ comb_w.T + comb_b)  (B, H)
  GRU step (r, z, n gates)  -> h_new                    (B, H)
  logp     = log_softmax(h_new @ out_w.T + out_b)       (B, V)

Sharding: data-parallel over batch B (front: embedding/attention/GRU),
tensor-parallel over vocab V for the output projection with a sharded
log_softmax (AllGather of h_new, AllReduce of sum-exp).

Layout convention on device: matmuls compute out[M,N] = lhsT[K,M].T @ rhs[K,N].
Activations are kept in [batch, feature] layout; the lhsT (stationary) operand
is the transposed activation [feature, batch], produced either by host-side
pre-transposition (inputs) or on-device PE transposes (intermediates).
Weight streams use wide per-k-chunk tiles (one DMA per 128-row chunk) with
k-outer / n-inner matmul loops accumulating into per-n PSUM banks.
"""

import sys

import numpy as np

sys.path.insert(0, "/opt/trn_rl_repo")

from concourse import bass, bacc, mybir, tile  # noqa: E402
from concourse.bass_utils import run_bass_kernel_spmd  # noqa: E402

H, V, L, B = 1024, 32000, 128, 256
NCORES = 8
BS = B // NCORES          # 32 batch rows per core
VS = V // NCORES          # 4000 vocab rows per core
KH = H // 128             # 8 hidden k-chunks
NV = 8                    # vocab n-chunks per core
VC = VS // NV             # 500 columns per vocab chunk
GRU_NC = 3 * H // 512     # 6 n-chunks for the 3H GRU gate matmuls
EB = 4                    # encoder batch rows per DMA tile

USE_BF16 = True


def build_graph(use_bf16=USE_BF16):
    f32 = mybir.dt.float32
    i32 = mybir.dt.int32
    dt = mybir.dt.bfloat16 if use_bf16 else f32

    nc = bacc.Bacc("TRN2", target_bir_lowering=False, debug=False,
                   num_devices=NCORES)

    # ---- kernel I/O ----
    ids_ext = nc.dram_tensor("ids", [1, BS], i32, kind="ExternalInput")
    emb_ext = nc.dram_tensor("emb", [V, H], f32, kind="ExternalInput")
    h0T_ext = nc.dram_tensor("h0t", [128, KH * BS], dt, kind="ExternalInput")
    h0_ext = nc.dram_tensor("h0", [BS, H], f32, kind="ExternalInput")
    # encoder outputs packed as enc[l, B_s*1024 + h] = enc_orig[b, l, h]
    enc_ext = nc.dram_tensor("enc", [L, BS * H], dt, kind="ExternalInput")
    # attn_w.T packed as awT[p, 128*k + l] = attn_w[l, 128*k + p]
    awT_ext = nc.dram_tensor("attn_wt", [128, 2 * KH * L], dt,
                             kind="ExternalInput")
    ab_ext = nc.dram_tensor("attn_b", [1, L], f32, kind="ExternalInput")
    cwT_ext = nc.dram_tensor("comb_wt", [2 * H, H], dt, kind="ExternalInput")
    cb_ext = nc.dram_tensor("comb_b", [1, H], f32, kind="ExternalInput")
    wihT_ext = nc.dram_tensor("w_iht", [H, 3 * H], dt, kind="ExternalInput")
    bih_ext = nc.dram_tensor("b_ih", [1, 3 * H], f32, kind="ExternalInput")
    whhT_ext = nc.dram_tensor("w_hht", [H, 3 * H], dt, kind="ExternalInput")
    bhh_ext = nc.dram_tensor("b_hh", [1, 3 * H], f32, kind="ExternalInput")
    owT_ext = nc.dram_tensor("out_wt", [H, VS], dt, kind="ExternalInput")
    ob_ext = nc.dram_tensor("out_b", [1, VS], f32, kind="ExternalInput")
    ident_ext = nc.dram_tensor("ident", [128, 128], f32, kind="ExternalInput")

    logp_ext = nc.dram_tensor("logp", [B, VS], f32, kind="ExternalOutput")
    hnew_ext = nc.dram_tensor("hnew", [BS, H], f32, kind="ExternalOutput")
    awout_ext = nc.dram_tensor("awout", [BS, L], f32, kind="ExternalOutput")

    with tile.TileContext(nc) as tc:
        from contextlib import ExitStack
        ctx = ExitStack()
        with ctx:
            const = ctx.enter_context(tc.tile_pool(name="const", bufs=1))
            act = ctx.enter_context(tc.tile_pool(name="act", bufs=1))
            wt = ctx.enter_context(tc.tile_pool(name="wt", bufs=2))
            dram = ctx.enter_context(tc.tile_pool(name="dram", bufs=1,
                                                  space="DRAM"))
            pp = ctx.enter_context(tc.tile_pool(name="pp", bufs=8,
                                                space="PSUM"))

            def psum(name):
                return pp.tile([BS, 512], f32, name=name, tag="ps")

            def psum128(name):
                return pp.tile([128, 512], f32, name=name, tag="ps")

            # ---- constants / small inputs ----
            ident = const.tile([128, 128], f32)
            nc.sync.dma_start(ident[:], ident_ext[:])
            ones = const.tile([1, 128], f32)
            nc.vector.memset(ones[:], 1.0)

            def bias_tile(ext, lo, n):
                bt = wt.tile([1, n], f32, name="btile", tag="btile", bufs=2)
                nc.sync.dma_start(bt[:], ext[0:1, lo:lo + n])
                return bt

            h0T_sb = const.tile([128, KH * BS], dt)
            nc.sync.dma_start(h0T_sb[:], h0T_ext[:])
            h0_sb = const.tile([BS, H], f32)
            nc.sync.dma_start(h0_sb[:], h0_ext[:])

            ident_bf = const.tile([128, 128], mybir.dt.bfloat16)
            nc.vector.tensor_copy(ident_bf[:], ident[:])

            def pe_transpose(dst_ap, src_ap, p, m, bf=False):
                """dst[m,p] (SBUF, any dtype) = src[p,m].T via PE; p=src parts."""
                if bf:
                    pt = pp.tile([128, 128], mybir.dt.bfloat16, name="ptb",
                                 tag="ps")
                    idn = ident_bf
                else:
                    pt = psum128("pt")
                    idn = ident
                nc.tensor.transpose(pt[:m, :p], src_ap, idn[:p, :p])
                nc.vector.tensor_copy(dst_ap, pt[:m, :p])

            # ---- embedding gather (data-dependent row DMAs, gpsimd) ----
            ids_sb = const.tile([1, BS], i32)
            emb_rows = act.tile([BS, H], f32)
            g_engines = [nc.gpsimd, nc.scalar]
            g_etypes = [mybir.EngineType.Pool, mybir.EngineType.Activation]
            g_sems = [nc.alloc_semaphore(f"gather_sem{e}") for e in range(2)]
            g_cnt = [0, 0]
            with tc.tile_critical():
                nc.gpsimd.dma_start(ids_sb[:], ids_ext[:]).then_inc(g_sems[0],
                                                                    16)
                for eng in g_engines:
                    eng.wait_ge(g_sems[0], 16)
                for i in range(BS):
                    ei = i % 2
                    eng = g_engines[ei]
                    reg = nc.alloc_register(g_etypes[ei], f"embidx{i}")
                    eng.load(reg, ids_sb[0:1, i:i + 1])
                    sv = nc.snap(reg, donate=True, min_val=0, max_val=V - 1)
                    eng.dma_start(
                        out=emb_rows[i:i + 1, :],
                        in_=emb_ext[bass.ds(sv, 1), :],
                    ).then_inc(g_sems[ei], 16)
                    g_cnt[ei] += 1
                nc.gpsimd.wait_ge(g_sems[0], 16 * (1 + g_cnt[0]))
                nc.scalar.wait_ge(g_sems[1], 16 * g_cnt[1])

            # ---- GRU gh = h0 @ w_hh.T + b_hh (independent of the gather,
            #      runs first so PE has dense work while the gather runs) ----
            gh_sb = act.tile([BS, 3 * H], f32)
            pgs = [psum(f"pgh{n}") for n in range(GRU_NC)]
            for k in range(KH):
                gwt = wt.tile([128, 3 * H], dt, name="gwt", tag="w3072",
                              bufs=2)
                nc.sync.dma_start(gwt[:], whhT_ext[128 * k:128 * (k + 1), :])
                for n in range(GRU_NC):
                    nc.tensor.matmul(pgs[n][:], h0T_sb[:, BS * k:BS * (k + 1)],
                                     gwt[:, 512 * n:512 * (n + 1)],
                                     start=(k == 0), stop=False,
                                     skip_group_check=True)
            for n in range(GRU_NC):
                gb = bias_tile(bhh_ext, 512 * n, 512)
                nc.tensor.matmul(pgs[n][:], ones[0:1, :BS], gb[:],
                                 start=False, stop=True, skip_group_check=True)
                nc.vector.tensor_copy(gh_sb[:, 512 * n:512 * (n + 1)],
                                      pgs[n][:])

            # ---- embT ----
            embT = act.tile([128, KH * BS], dt)
            for k in range(KH):
                pe_transpose(embT[:, BS * k:BS * (k + 1)],
                             emb_rows[:, 128 * k:128 * (k + 1)], BS, 128)

            # ---- attn_logits = [emb, h0] @ attn_w.T + attn_b ----
            awt = wt.tile([128, 2 * KH * L], dt, name="awt", tag="wattn",
                          bufs=1)
            nc.sync.dma_start(awt[:], awT_ext[:])
            pa = psum("pa")
            for k in range(2 * KH):
                lhsT = (embT[:, BS * k:BS * (k + 1)] if k < KH
                        else h0T_sb[:, BS * (k - KH):BS * (k - KH + 1)])
                nc.tensor.matmul(pa[:, :L], lhsT,
                                 awt[:, L * k:L * (k + 1)],
                                 start=(k == 0), stop=False)
            ab_sb = bias_tile(ab_ext, 0, L)
            nc.tensor.matmul(pa[:, :L], ones[0:1, :BS], ab_sb[:], start=False,
                             stop=True)

            # ---- softmax over L -> aw; awT; block-diag bd ----
            aw_pre = act.tile([BS, L], f32)
            nc.vector.tensor_copy(aw_pre[:], pa[:, :L])
            nmx = act.tile([BS, 1], f32)
            nc.vector.tensor_reduce(nmx[:], aw_pre[:], mybir.AxisListType.X,
                                    mybir.AluOpType.max, negate=True)
            aw_exp = act.tile([BS, L], f32)
            se = act.tile([BS, 1], f32)
            nc.scalar.activation(aw_exp[:], aw_pre[:],
                                 mybir.ActivationFunctionType.Exp,
                                 bias=nmx[:], scale=1.0, accum_out=se[:])
            rse = act.tile([BS, 1], f32)
            nc.vector.reciprocal(rse[:], se[:])
            aw_sb = act.tile([BS, L], f32)
            nc.vector.tensor_scalar_mul(aw_sb[:], aw_exp[:], rse[:])
            nc.sync.dma_start(awout_ext[:], aw_sb[:])

            awT_sb = act.tile([128, BS], dt)
            pe_transpose(awT_sb[:], aw_sb[:], BS, 128)
            # block-diagonal lhsT: bd[:, 32*b + b] = awT[:, b]
            bd = act.tile([128, BS * BS], dt)
            nc.vector.memset(bd[:], 0.0)
            for b in range(BS):
                nc.vector.tensor_copy(bd[:, 33 * b:33 * b + 1],
                                      awT_sb[:, b:b + 1])

            # ---- attn_applied[b,:] = aw[b,:] @ enc[b] ----
            patt = [psum("patt0"), psum("patt1")]
            for g in range(BS // EB):
                et = wt.tile([128, EB * H], dt, name="enc", tag="enc", bufs=3)
                nc.sync.dma_start(et[:], enc_ext[:, EB * H * g:
                                                 EB * H * (g + 1)])
                for bb in range(EB):
                    b = EB * g + bb
                    for c in range(2):
                        nc.tensor.matmul(
                            patt[c][:],
                            bd[:, BS * b:BS * (b + 1)],
                            et[:, H * bb + 512 * c:H * bb + 512 * (c + 1)],
                            start=(b == 0), stop=(b == BS - 1),
                            skip_group_check=True)
            att_sb = act.tile([BS, H], f32)
            for c in range(2):
                nc.vector.tensor_copy(att_sb[:, 512 * c:512 * (c + 1)],
                                      patt[c][:])

            # ---- attT ----
            attT = act.tile([128, KH * BS], dt)
            for k in range(KH):
                pe_transpose(attT[:, BS * k:BS * (k + 1)],
                             att_sb[:, 128 * k:128 * (k + 1)], BS, 128)

            # ---- x = relu([emb, att] @ comb_w.T + comb_b) ----
            x_sb = act.tile([BS, H], f32)
            pxs = [psum("px0"), psum("px1")]
            for k in range(2 * KH):
                cwt = wt.tile([128, H], dt, name="cwt", tag="w1024", bufs=2)
                nc.sync.dma_start(cwt[:], cwT_ext[128 * k:128 * (k + 1), :])
                lhsT = (embT[:, BS * k:BS * (k + 1)] if k < KH
                        else attT[:, BS * (k - KH):BS * (k - KH + 1)])
                for n in range(2):
                    nc.tensor.matmul(pxs[n][:], lhsT,
                                     cwt[:, 512 * n:512 * (n + 1)],
                                     start=(k == 0), stop=False,
                                     skip_group_check=True)
            for n in range(2):
                cb_sb = bias_tile(cb_ext, 512 * n, 512)
                nc.tensor.matmul(pxs[n][:], ones[0:1, :BS], cb_sb[:],
                                 start=False, stop=True, skip_group_check=True)
                nc.scalar.activation(x_sb[:, 512 * n:512 * (n + 1)],
                                     pxs[n][:],
                                     mybir.ActivationFunctionType.Relu)

            # ---- xT ----
            xT = act.tile([128, KH * BS], dt)
            for k in range(KH):
                pe_transpose(xT[:, BS * k:BS * (k + 1)],
                             x_sb[:, 128 * k:128 * (k + 1)], BS, 128)

            # ---- gx = x @ w_ih.T + b_ih ----
            gx_sb = act.tile([BS, 3 * H], f32)
            pgx = [psum(f"pgx{n}") for n in range(GRU_NC)]
            for k in range(KH):
                gwt2 = wt.tile([128, 3 * H], dt, name="gwt2", tag="w3072",
                               bufs=2)
                nc.sync.dma_start(gwt2[:], wihT_ext[128 * k:128 * (k + 1), :])
                for n in range(GRU_NC):
                    nc.tensor.matmul(pgx[n][:], xT[:, BS * k:BS * (k + 1)],
                                     gwt2[:, 512 * n:512 * (n + 1)],
                                     start=(k == 0), stop=False,
                                     skip_group_check=True)
            for n in range(GRU_NC):
                gb2 = bias_tile(bih_ext, 512 * n, 512)
                nc.tensor.matmul(pgx[n][:], ones[0:1, :BS], gb2[:],
                                 start=False, stop=True, skip_group_check=True)
                nc.vector.tensor_copy(gx_sb[:, 512 * n:512 * (n + 1)],
                                      pgx[n][:])

            # ---- GRU elementwise -> h_new ----
            t1 = act.tile([BS, H], f32, tag="gtmp", bufs=2)
            r_sb = act.tile([BS, H], f32)
            nc.vector.tensor_add(t1[:], gx_sb[:, 0:H], gh_sb[:, 0:H])
            nc.scalar.activation(r_sb[:], t1[:],
                                 mybir.ActivationFunctionType.Sigmoid)
            t2 = act.tile([BS, H], f32, tag="gtmp", bufs=2)
            z_sb = act.tile([BS, H], f32)
            nc.vector.tensor_add(t2[:], gx_sb[:, H:2 * H], gh_sb[:, H:2 * H])
            nc.scalar.activation(z_sb[:], t2[:],
                                 mybir.ActivationFunctionType.Sigmoid)
            t3 = act.tile([BS, H], f32, tag="gtmp", bufs=2)
            n_sb = act.tile([BS, H], f32)
            nc.vector.tensor_mul(t3[:], r_sb[:], gh_sb[:, 2 * H:3 * H])
            nc.vector.tensor_add(t3[:], t3[:], gx_sb[:, 2 * H:3 * H])
            nc.scalar.activation(n_sb[:], t3[:],
                                 mybir.ActivationFunctionType.Tanh)
            t4 = act.tile([BS, H], f32, tag="gtmp", bufs=2)
            hnew_sb = act.tile([BS, H], f32)
            nc.vector.tensor_sub(t4[:], h0_sb[:], n_sb[:])
            nc.vector.tensor_mul(t4[:], z_sb[:], t4[:])
            nc.vector.tensor_add(hnew_sb[:], n_sb[:], t4[:])
            nc.sync.dma_start(hnew_ext[:], hnew_sb[:])

            # ---- AllGather h_new across cores (bf16 payload) ----
            hnew_bf = act.tile([BS, H], dt)
            nc.vector.tensor_copy(hnew_bf[:], hnew_sb[:])
            hnew_bounce = dram.tile([BS, H], dt)
            h_all_dram = dram.tile([B, H], dt, addr_space="Shared")
            nc.sync.dma_start(hnew_bounce[:], hnew_bf[:])
            nc.gpsimd.collective_compute(
                "AllGather", mybir.AluOpType.bypass,
                replica_groups=[list(range(NCORES))],
                ins=[hnew_bounce[:]], outs=[h_all_dram[:]])
            h_allT = []
            for m in range(2):
                ha = act.tile([128, H], dt, tag="h_all", name=f"h_all{m}")
                nc.sync.dma_start(ha[:], h_all_dram[128 * m:128 * (m + 1), :])
                for k in range(KH):
                    if m == 0:
                        h_allT.append(act.tile([128, 2 * 128], dt,
                                               name=f"h_allT{k}",
                                               tag=f"h_allT{k}"))
                    pe_transpose(h_allT[k][:, 128 * m:128 * (m + 1)],
                                 ha[:, 128 * k:128 * (k + 1)], 128, 128,
                                 bf=use_bf16)

            # ---- logits = h_all @ out_w.T + out_b; exp partial sums ----
            # vocab in 2 halves of 2000; k-outer so each out_w k-chunk is
            # DMA'd once and serves both batch halves m.
            HNV = NV // 2                  # 4 n-chunks per half
            logits = [act.tile([128, VS], dt, tag=f"logits{m}",
                               name=f"logits{m}") for m in range(2)]
            lsums = act.tile([128, 2 * NV], f32)
            for half in range(2):
                pls = [pp.tile([128, VC], f32, name=f"pl{m}_{n}", tag="ps")
                       for m in range(2) for n in range(HNV)]
                for k in range(KH):
                    owt = wt.tile([128, VS // 2], dt, name="owt", tag="w2000",
                                  bufs=3)
                    nc.sync.dma_start(
                        owt[:], owT_ext[128 * k:128 * (k + 1),
                                        (VS // 2) * half:(VS // 2) * (half + 1)])
                    for m in range(2):
                        for n in range(HNV):
                            nc.tensor.matmul(
                                pls[HNV * m + n][:],
                                h_allT[k][:, 128 * m:128 * (m + 1)],
                                owt[:, VC * n:VC * (n + 1)],
                                start=(k == 0), stop=False,
                                skip_group_check=True)
                for m in range(2):
                    for n in range(HNV):
                        nv = HNV * half + n
                        ob_sb = bias_tile(ob_ext, VC * nv, VC)
                        nc.tensor.matmul(pls[HNV * m + n][:], ones[:],
                                         ob_sb[:], start=False, stop=True,
                                         skip_group_check=True)
                        nc.vector.tensor_copy(
                            logits[m][:, VC * nv:VC * (nv + 1)],
                            pls[HNV * m + n][:])
                        esc = act.tile([128, VC], f32, tag="escratch",
                                       name="esc")
                        nc.scalar.activation(
                            esc[:], logits[m][:, VC * nv:VC * (nv + 1)],
                            mybir.ActivationFunctionType.Exp,
                            accum_out=lsums[:, NV * m + nv:NV * m + nv + 1])

            # ---- global log-sum-exp via AllReduce ----
            lsum_sb = act.tile([128, 2], f32)
            for m in range(2):
                nc.vector.tensor_reduce(lsum_sb[:, m:m + 1],
                                        lsums[:, NV * m:NV * (m + 1)],
                                        mybir.AxisListType.X,
                                        mybir.AluOpType.add)
            lsumT_sb = act.tile([2, 128], f32)
            pe_transpose(lsumT_sb[:], lsum_sb[:], 128, 2)
            lsumT_dram = dram.tile([2, 128], f32)
            gsumT_dram = dram.tile([2, 128], f32, addr_space="Shared")
            nc.sync.dma_start(lsumT_dram[:], lsumT_sb[:])
            nc.gpsimd.collective_compute(
                "AllReduce", mybir.AluOpType.add,
                replica_groups=[list(range(NCORES))],
                ins=[lsumT_dram[:]], outs=[gsumT_dram[:]])
            gsumT_sb = act.tile([2, 128], f32)
            nc.sync.dma_start(gsumT_sb[:], gsumT_dram[:])
            loggT_sb = act.tile([2, 128], f32)
            nc.scalar.activation(loggT_sb[:], gsumT_sb[:],
                                 mybir.ActivationFunctionType.Ln)
            logg_sb = act.tile([128, 2], f32)
            pe_transpose(logg_sb[:], loggT_sb[:], 2, 128)

            # ---- logp = logits - log(gsum); write out chunked ----
            for m in range(2):
                for n in range(NV):
                    lp = act.tile([128, VC], f32, tag="lpstage", name="lp",
                                  bufs=4)
                    nc.vector.tensor_scalar_sub(
                        lp[:], logits[m][:, VC * n:VC * (n + 1)],
                        logg_sb[:, m:m + 1])
                    nc.sync.dma_start(
                        logp_ext[128 * m:128 * (m + 1), VC * n:VC * (n + 1)],
                        lp[:])

    nc.compile()
    return nc


def stage_inputs(input_ids, hidden, encoder_outputs, emb_table,
                 attn_w, attn_b, comb_w, comb_b,
                 w_ih, b_ih, w_hh, b_hh, out_w, out_b, use_bf16=USE_BF16):
    if use_bf16:
        import ml_dtypes
        np_dt = ml_dtypes.bfloat16
    else:
        np_dt = np.float32
    f32 = np.float32

    def cvt(x, dtype):
        return np.ascontiguousarray(np.asarray(x), dtype=dtype)

    ids = np.asarray(input_ids).astype(np.int32).reshape(1, B)
    h0 = cvt(hidden, f32).reshape(B, H)
    enc = cvt(encoder_outputs, np_dt)
    emb = cvt(emb_table, f32)
    # attn_w.T (2H, L) packed into (128, 16*L): [p, L*k + l] = attn_w.T[128k+p, l]
    awT = np.asarray(attn_w).T.astype(np_dt)  # (2H, L)
    awT_packed = np.ascontiguousarray(
        awT.reshape(2 * KH, 128, L).transpose(1, 0, 2).reshape(128, 2 * KH * L))
    ab = cvt(attn_b, f32).reshape(1, L)
    cwT = cvt(np.asarray(comb_w).T, np_dt)
    cb = cvt(comb_b, f32).reshape(1, H)
    wihT = cvt(np.asarray(w_ih).T, np_dt)
    bih = cvt(b_ih, f32).reshape(1, 3 * H)
    whhT = cvt(np.asarray(w_hh).T, np_dt)
    bhh = cvt(b_hh, f32).reshape(1, 3 * H)
    owT_full = np.asarray(out_w).T  # (H, V)
    ob = cvt(out_b, f32).reshape(1, V)
    ident = np.eye(128, dtype=f32)

    in_maps = []
    for j in range(NCORES):
        bsl = slice(BS * j, BS * (j + 1))
        vsl = slice(VS * j, VS * (j + 1))
        h0_j = np.ascontiguousarray(h0[bsl])
        # packed transposed hidden: h0T[p, BS*k + b] = h0_j[b, 128k + p]
        h0T_j = np.ascontiguousarray(
            h0_j.T.reshape(KH, 128, BS).transpose(1, 0, 2).reshape(128, KH * BS),
            dtype=np_dt)
        # encoder shard packed as (L, BS*H): [l, 1024*b + h]
        enc_j = np.ascontiguousarray(
            enc[bsl].transpose(1, 0, 2).reshape(L, BS * H))
        in_maps.append({
            "ids": np.ascontiguousarray(ids[:, bsl]),
            "emb": emb,
            "h0t": h0T_j,
            "h0": h0_j,
            "enc": enc_j,
            "attn_wt": awT_packed,
            "attn_b": ab,
            "comb_wt": cwT,
            "comb_b": cb,
            "w_iht": wihT,
            "b_ih": bih,
            "w_hht": whhT,
            "b_hh": bhh,
            "out_wt": np.ascontiguousarray(owT_full[:, vsl], dtype=np_dt),
            "out_b": np.ascontiguousarray(ob[:, vsl]),
            "ident": ident,
        })
    return in_maps


def run(inputs, trace=False, trace_cores=None, use_bf16=USE_BF16):
    nc = build_graph(use_bf16)
    in_maps = stage_inputs(**inputs, use_bf16=use_bf16)
    res = run_bass_kernel_spmd(
        nc, in_maps, core_ids=list(range(NCORES)),
        trace=trace, trace_cores=trace_cores)
    r = res.results
    logp = np.concatenate([r[j]["logp"] for j in range(NCORES)], axis=1)
    hnew = np.concatenate([r[j]["hnew"] for j in range(NCORES)], axis=0)[None]
    aw = np.concatenate([r[j]["awout"] for j in range(NCORES)], axis=0)
    return (logp, hnew, aw), res


def kernel(**inputs):
    outs, _ = run(inputs, trace=False)
    return outs


# revision 8
# speedup vs baseline: 1.5798x; 1.1301x over previous
"""Trainium2 8-core SPMD Bass kernel for a single AttnDecoderRNN step.

Reference computation (per step):
  embedded = emb_table[input_ids]                       (1, B, H)
  attn_w'  = softmax([emb; h] @ attn_w.T + attn_b)      (B, L)
  attn_app = einsum('bl,blh->bh', attn_w', enc)         (B, H)
  x        = relu([emb; attn_app] @ comb_w.T + comb_b)  (B, H)
  GRU step (r, z, n gates)  -> h_new                    (B, H)
  logp     = log_softmax(h_new @ out_w.T + out_b)       (B, V)

Sharding: data-parallel over batch B (front: embedding/attention/GRU),
tensor-parallel over vocab V for the output projection with a sharded
log_softmax (AllGather of h_new, AllReduce of sum-exp).

Layout convention on device: matmuls compute out[M,N] = lhsT[K,M].T @ rhs[K,N].
Activations are kept in [batch, feature] layout; the lhsT (stationary) operand
is the transposed activation [feature, batch], produced either by host-side
pre-transposition (inputs) or on-device PE transposes (intermediates).
Weight streams use wide per-k-chunk tiles (one DMA per 128-row chunk) with
k-outer / n-inner matmul loops accumulating into per-n PSUM banks.
"""

import sys

import numpy as np

sys.path.insert(0, "/opt/trn_rl_repo")

from concourse import bass, bacc, mybir, tile  # noqa: E402
from concourse.bass_utils import run_bass_kernel_spmd  # noqa: E402

H, V, L, B = 1024, 32000, 128, 256
NCORES = 8
BS = B // NCORES          # 32 batch rows per core
VS = V // NCORES          # 4000 vocab rows per core
KH = H // 128             # 8 hidden k-chunks
NV = 8                    # vocab n-chunks per core
VC = VS // NV             # 500 columns per vocab chunk
GRU_NC = 3 * H // 512     # 6 n-chunks for the 3H GRU gate matmuls
EB = 4                    # encoder batch rows per DMA tile

USE_BF16 = True


def build_graph(use_bf16=USE_BF16):
    f32 = mybir.dt.float32
    i32 = mybir.dt.int32
    dt = mybir.dt.bfloat16 if use_bf16 else f32

    nc = bacc.Bacc("TRN2", target_bir_lowering=False, debug=False,
                   num_devices=NCORES)

    # ---- kernel I/O ----
    ids_ext = nc.dram_tensor("ids", [1, BS], i32, kind="ExternalInput")
    emb_ext = nc.dram_tensor("emb", [V, H], f32, kind="ExternalInput")
    h0T_ext = nc.dram_tensor("h0t", [128, KH * BS], dt, kind="ExternalInput")
    h0_ext = nc.dram_tensor("h0", [BS, H], f32, kind="ExternalInput")
    # encoder outputs packed as enc[l, B_s*1024 + h] = enc_orig[b, l, h]
    enc_ext = nc.dram_tensor("enc", [L, BS * H], dt, kind="ExternalInput")
    # attn_w.T packed as awT[p, 128*k + l] = attn_w[l, 128*k + p]
    awT_ext = nc.dram_tensor("attn_wt", [128, 2 * KH * L], dt,
                             kind="ExternalInput")
    ab_ext = nc.dram_tensor("attn_b", [1, L], dt, kind="ExternalInput")
    cwT_ext = nc.dram_tensor("comb_wt", [2 * H, H], dt, kind="ExternalInput")
    cb_ext = nc.dram_tensor("comb_b", [1, H], dt, kind="ExternalInput")
    wihT_ext = nc.dram_tensor("w_iht", [H, 3 * H], dt, kind="ExternalInput")
    bih_ext = nc.dram_tensor("b_ih", [1, 3 * H], dt, kind="ExternalInput")
    whhT_ext = nc.dram_tensor("w_hht", [H, 3 * H], dt, kind="ExternalInput")
    bhh_ext = nc.dram_tensor("b_hh", [1, 3 * H], dt, kind="ExternalInput")
    owT_ext = nc.dram_tensor("out_wt", [H, VS], dt, kind="ExternalInput")
    ob_ext = nc.dram_tensor("out_b", [1, VS], dt, kind="ExternalInput")
    ident_ext = nc.dram_tensor("ident", [128, 128], f32, kind="ExternalInput")

    logp_ext = nc.dram_tensor("logp", [B, VS], f32, kind="ExternalOutput")
    hnew_ext = nc.dram_tensor("hnew", [BS, H], f32, kind="ExternalOutput")
    awout_ext = nc.dram_tensor("awout", [BS, L], f32, kind="ExternalOutput")

    with tile.TileContext(nc) as tc:
        from contextlib import ExitStack
        ctx = ExitStack()
        with ctx:
            const = ctx.enter_context(tc.tile_pool(name="const", bufs=1))
            act = ctx.enter_context(tc.tile_pool(name="act", bufs=1))
            wt = ctx.enter_context(tc.tile_pool(name="wt", bufs=2))
            dram = ctx.enter_context(tc.tile_pool(name="dram", bufs=1,
                                                  space="DRAM"))
            pp = ctx.enter_context(tc.tile_pool(name="pp", bufs=8,
                                                space="PSUM"))

            def psum(name):
                return pp.tile([BS, 512], f32, name=name, tag="ps")

            def psum128(name):
                return pp.tile([128, 512], f32, name=name, tag="ps")

            # ---- constants / small inputs ----
            ident = const.tile([128, 128], f32)
            nc.sync.dma_start(ident[:], ident_ext[:])
            def bias_bcast(ext, n, name):
                bt = const.tile([128, n], dt, name=name)
                nc.scalar.dma_start(bt[:], ext[0:1, :].partition_broadcast(128))
                return bt

            ab_bc = bias_bcast(ab_ext, L, "ab_bc")
            cb_bc = bias_bcast(cb_ext, H, "cb_bc")
            bih_bc = bias_bcast(bih_ext, 3 * H, "bih_bc")
            bhh_bc = bias_bcast(bhh_ext, 3 * H, "bhh_bc")
            ob_bc = bias_bcast(ob_ext, VS, "ob_bc")

            h0T_sb = const.tile([128, KH * BS], dt)
            nc.sync.dma_start(h0T_sb[:], h0T_ext[:])
            h0_sb = const.tile([BS, H], f32)
            nc.sync.dma_start(h0_sb[:], h0_ext[:])

            ident_bf = const.tile([128, 128], mybir.dt.bfloat16)
            nc.vector.tensor_copy(ident_bf[:], ident[:])

            def pe_transpose(dst_ap, src_ap, p, m, bf=False):
                """dst[m,p] (SBUF, any dtype) = src[p,m].T via PE; p=src parts."""
                if bf:
                    pt = pp.tile([128, 128], mybir.dt.bfloat16, name="ptb",
                                 tag="ps")
                    idn = ident_bf
                else:
                    pt = psum128("pt")
                    idn = ident
                nc.tensor.transpose(pt[:m, :p], src_ap, idn[:p, :p])
                nc.vector.tensor_copy(dst_ap, pt[:m, :p])

            # ---- embedding gather (data-dependent row DMAs, gpsimd) ----
            ids_sb = const.tile([1, BS], i32)
            emb_rows = act.tile([BS, H], f32)
            g_engines = [nc.gpsimd, nc.scalar]
            g_etypes = [mybir.EngineType.Pool, mybir.EngineType.Activation]
            g_sems = [nc.alloc_semaphore(f"gather_sem{e}") for e in range(2)]
            g_cnt = [0, 0]
            with tc.tile_critical():
                nc.gpsimd.dma_start(ids_sb[:], ids_ext[:]).then_inc(g_sems[0],
                                                                    16)
                for eng in g_engines:
                    eng.wait_ge(g_sems[0], 16)
                for i in range(BS):
                    ei = i % 2
                    eng = g_engines[ei]
                    reg = nc.alloc_register(g_etypes[ei], f"embidx{i}")
                    eng.load(reg, ids_sb[0:1, i:i + 1])
                    sv = nc.snap(reg, donate=True, min_val=0, max_val=V - 1)
                    eng.dma_start(
                        out=emb_rows[i:i + 1, :],
                        in_=emb_ext[bass.ds(sv, 1), :],
                    ).then_inc(g_sems[ei], 16)
                    g_cnt[ei] += 1
                nc.gpsimd.wait_ge(g_sems[0], 16 * (1 + g_cnt[0]))
                nc.scalar.wait_ge(g_sems[1], 16 * g_cnt[1])

            # ---- GRU gh = h0 @ w_hh.T + b_hh (independent of the gather,
            #      runs first so PE has dense work while the gather runs) ----
            gh_sb = act.tile([BS, 3 * H], dt)
            pgs = [psum(f"pgh{n}") for n in range(GRU_NC)]
            for k in range(KH):
                gwt = wt.tile([128, 3 * H], dt, name="gwt", tag="w3072",
                              bufs=2)
                nc.sync.dma_start(gwt[:], whhT_ext[128 * k:128 * (k + 1), :])
                for n in range(GRU_NC):
                    nc.tensor.matmul(pgs[n][:], h0T_sb[:, BS * k:BS * (k + 1)],
                                     gwt[:, 512 * n:512 * (n + 1)],
                                     start=(k == 0), stop=(k == KH - 1),
                                     skip_group_check=True)
            for n in range(GRU_NC):
                nc.vector.tensor_add(gh_sb[:, 512 * n:512 * (n + 1)],
                                     pgs[n][:],
                                     bhh_bc[:BS, 512 * n:512 * (n + 1)])

            # ---- embT ----
            embT = act.tile([128, KH * BS], dt)
            for k in range(KH):
                pe_transpose(embT[:, BS * k:BS * (k + 1)],
                             emb_rows[:, 128 * k:128 * (k + 1)], BS, 128)

            # ---- attn_logits = [emb, h0] @ attn_w.T + attn_b ----
            awt = wt.tile([128, 2 * KH * L], dt, name="awt", tag="wattn",
                          bufs=1)
            nc.sync.dma_start(awt[:], awT_ext[:])
            pa = psum("pa")
            for k in range(2 * KH):
                lhsT = (embT[:, BS * k:BS * (k + 1)] if k < KH
                        else h0T_sb[:, BS * (k - KH):BS * (k - KH + 1)])
                nc.tensor.matmul(pa[:, :L], lhsT,
                                 awt[:, L * k:L * (k + 1)],
                                 start=(k == 0), stop=(k == 2 * KH - 1))

            # ---- softmax over L -> aw; awT; block-diag bd ----
            aw_pre = act.tile([BS, L], f32)
            nc.vector.tensor_add(aw_pre[:], pa[:, :L], ab_bc[:BS, :])
            nmx = act.tile([BS, 1], f32)
            nc.vector.tensor_reduce(nmx[:], aw_pre[:], mybir.AxisListType.X,
                                    mybir.AluOpType.max, negate=True)
            aw_exp = act.tile([BS, L], f32)
            se = act.tile([BS, 1], f32)
            nc.scalar.activation(aw_exp[:], aw_pre[:],
                                 mybir.ActivationFunctionType.Exp,
                                 bias=nmx[:], scale=1.0, accum_out=se[:])
            rse = act.tile([BS, 1], f32)
            nc.vector.reciprocal(rse[:], se[:])
            aw_sb = act.tile([BS, L], f32)
            nc.vector.tensor_scalar_mul(aw_sb[:], aw_exp[:], rse[:])
            nc.scalar.dma_start(awout_ext[:], aw_sb[:])

            awT_sb = act.tile([128, BS], dt)
            pe_transpose(awT_sb[:], aw_sb[:], BS, 128)
            # block-diagonal lhsT: bd[:, 32*b + b] = awT[:, b]
            bd = act.tile([128, BS * BS], dt)
            nc.vector.memset(bd[:], 0.0)
            for b in range(BS):
                nc.vector.tensor_copy(bd[:, 33 * b:33 * b + 1],
                                      awT_sb[:, b:b + 1])

            # ---- attn_applied[b,:] = aw[b,:] @ enc[b] ----
            patt = [psum("patt0"), psum("patt1")]
            for g in range(BS // EB):
                et = wt.tile([128, EB * H], dt, name="enc", tag="enc", bufs=3)
                nc.gpsimd.dma_start(et[:], enc_ext[:, EB * H * g:
                                                 EB * H * (g + 1)])
                for bb in range(EB):
                    b = EB * g + bb
                    for c in range(2):
                        nc.tensor.matmul(
                            patt[c][:],
                            bd[:, BS * b:BS * (b + 1)],
                            et[:, H * bb + 512 * c:H * bb + 512 * (c + 1)],
                            start=(b == 0), stop=(b == BS - 1),
                            skip_group_check=True)
            att_sb = act.tile([BS, H], dt)
            for c in range(2):
                nc.vector.tensor_copy(att_sb[:, 512 * c:512 * (c + 1)],
                                      patt[c][:])

            # ---- attT ----
            attT = act.tile([128, KH * BS], dt)
            for k in range(KH):
                pe_transpose(attT[:, BS * k:BS * (k + 1)],
                             att_sb[:, 128 * k:128 * (k + 1)], BS, 128,
                             bf=use_bf16)

            # ---- x = relu([emb, att] @ comb_w.T + comb_b) ----
            x_sb = act.tile([BS, H], dt)
            pxs = [psum("px0"), psum("px1")]
            for k in range(2 * KH):
                cwt = wt.tile([128, H], dt, name="cwt", tag="w1024", bufs=2)
                nc.sync.dma_start(cwt[:], cwT_ext[128 * k:128 * (k + 1), :])
                lhsT = (embT[:, BS * k:BS * (k + 1)] if k < KH
                        else attT[:, BS * (k - KH):BS * (k - KH + 1)])
                for n in range(2):
                    nc.tensor.matmul(pxs[n][:], lhsT,
                                     cwt[:, 512 * n:512 * (n + 1)],
                                     start=(k == 0), stop=(k == 2 * KH - 1),
                                     skip_group_check=True)
            for n in range(2):
                nc.vector.tensor_add(x_sb[:, 512 * n:512 * (n + 1)],
                                     pxs[n][:],
                                     cb_bc[:BS, 512 * n:512 * (n + 1)])
                nc.vector.tensor_scalar_max(x_sb[:, 512 * n:512 * (n + 1)],
                                            x_sb[:, 512 * n:512 * (n + 1)],
                                            0.0)

            # ---- xT ----
            xT = act.tile([128, KH * BS], dt)
            for k in range(KH):
                pe_transpose(xT[:, BS * k:BS * (k + 1)],
                             x_sb[:, 128 * k:128 * (k + 1)], BS, 128,
                             bf=use_bf16)

            # ---- gx = x @ w_ih.T + b_ih ----
            gx_sb = act.tile([BS, 3 * H], dt)
            pgx = [psum(f"pgx{n}") for n in range(GRU_NC)]
            for k in range(KH):
                gwt2 = wt.tile([128, 3 * H], dt, name="gwt2", tag="w3072",
                               bufs=2)
                nc.sync.dma_start(gwt2[:], wihT_ext[128 * k:128 * (k + 1), :])
                for n in range(GRU_NC):
                    nc.tensor.matmul(pgx[n][:], xT[:, BS * k:BS * (k + 1)],
                                     gwt2[:, 512 * n:512 * (n + 1)],
                                     start=(k == 0), stop=(k == KH - 1),
                                     skip_group_check=True)
            for n in range(GRU_NC):
                nc.vector.tensor_add(gx_sb[:, 512 * n:512 * (n + 1)],
                                     pgx[n][:],
                                     bih_bc[:BS, 512 * n:512 * (n + 1)])

            # ---- GRU elementwise -> h_new ----
            t1 = act.tile([BS, H], dt, tag="gtmp", bufs=2)
            r_sb = act.tile([BS, H], dt)
            nc.vector.tensor_add(t1[:], gx_sb[:, 0:H], gh_sb[:, 0:H])
            nc.scalar.activation(r_sb[:], t1[:],
                                 mybir.ActivationFunctionType.Sigmoid)
            t2 = act.tile([BS, H], dt, tag="gtmp", bufs=2)
            z_sb = act.tile([BS, H], dt)
            nc.vector.tensor_add(t2[:], gx_sb[:, H:2 * H], gh_sb[:, H:2 * H])
            nc.scalar.activation(z_sb[:], t2[:],
                                 mybir.ActivationFunctionType.Sigmoid)
            t3 = act.tile([BS, H], dt, tag="gtmp", bufs=2)
            n_sb = act.tile([BS, H], dt)
            nc.vector.tensor_mul(t3[:], r_sb[:], gh_sb[:, 2 * H:3 * H])
            nc.vector.tensor_add(t3[:], t3[:], gx_sb[:, 2 * H:3 * H])
            nc.scalar.activation(n_sb[:], t3[:],
                                 mybir.ActivationFunctionType.Tanh)
            t4 = act.tile([BS, H], f32, tag="gtmp2", bufs=1)
            hnew_sb = act.tile([BS, H], f32)
            nc.vector.tensor_sub(t4[:], h0_sb[:], n_sb[:])
            nc.vector.tensor_mul(t4[:], z_sb[:], t4[:])
            nc.vector.tensor_add(hnew_sb[:], n_sb[:], t4[:])
            nc.scalar.dma_start(hnew_ext[:], hnew_sb[:])

            # ---- AllGather h_new across cores (bf16 payload) ----
            hnew_bf = act.tile([BS, H], dt)
            nc.vector.tensor_copy(hnew_bf[:], hnew_sb[:])
            hnew_bounce = dram.tile([BS, H], dt)
            h_all_dram = dram.tile([B, H], dt, addr_space="Shared")
            nc.sync.dma_start(hnew_bounce[:], hnew_bf[:])
            nc.gpsimd.collective_compute(
                "AllGather", mybir.AluOpType.bypass,
                replica_groups=[list(range(NCORES))],
                ins=[hnew_bounce[:]], outs=[h_all_dram[:]])
            h_allT = []
            for m in range(2):
                ha = act.tile([128, H], dt, tag="h_all", name=f"h_all{m}")
                nc.gpsimd.dma_start(ha[:], h_all_dram[128 * m:128 * (m + 1), :])
                for k in range(KH):
                    if m == 0:
                        h_allT.append(act.tile([128, 2 * 128], dt,
                                               name=f"h_allT{k}",
                                               tag=f"h_allT{k}"))
                    pe_transpose(h_allT[k][:, 128 * m:128 * (m + 1)],
                                 ha[:, 128 * k:128 * (k + 1)], 128, 128,
                                 bf=use_bf16)

            # ---- logits = h_all @ out_w.T + out_b; exp partial sums ----
            # vocab in 2 halves of 2000; k-outer so each out_w k-chunk is
            # DMA'd once and serves both batch halves m.
            HNV = NV // 2                  # 4 n-chunks per half
            logits = [act.tile([128, VS], dt, tag=f"logits{m}",
                               name=f"logits{m}") for m in range(2)]
            lsums = act.tile([128, 2 * NV], f32)
            for half in range(2):
                pls = [pp.tile([128, VC], f32, name=f"pl{m}_{n}", tag="ps")
                       for m in range(2) for n in range(HNV)]
                for k in range(KH):
                    owt = wt.tile([128, VS // 2], dt, name="owt", tag="w2000",
                                  bufs=6)
                    nc.scalar.dma_start(
                        owt[:], owT_ext[128 * k:128 * (k + 1),
                                        (VS // 2) * half:(VS // 2) * (half + 1)])
                    for m in range(2):
                        for n in range(HNV):
                            nc.tensor.matmul(
                                pls[HNV * m + n][:],
                                h_allT[k][:, 128 * m:128 * (m + 1)],
                                owt[:, VC * n:VC * (n + 1)],
                                start=(k == 0), stop=(k == KH - 1),
                                skip_group_check=True)
                for m in range(2):
                    for n in range(HNV):
                        nv = HNV * half + n
                        nc.vector.tensor_add(
                            logits[m][:, VC * nv:VC * (nv + 1)],
                            pls[HNV * m + n][:],
                            ob_bc[:, VC * nv:VC * (nv + 1)])
                        esc = act.tile([128, VC], f32, tag="escratch",
                                       name="esc")
                        nc.scalar.activation(
                            esc[:], logits[m][:, VC * nv:VC * (nv + 1)],
                            mybir.ActivationFunctionType.Exp,
                            accum_out=lsums[:, NV * m + nv:NV * m + nv + 1])

            # ---- global log-sum-exp via AllReduce ----
            lsum_sb = act.tile([128, 2], f32)
            for m in range(2):
                nc.vector.tensor_reduce(lsum_sb[:, m:m + 1],
                                        lsums[:, NV * m:NV * (m + 1)],
                                        mybir.AxisListType.X,
                                        mybir.AluOpType.add)
            lsumT_sb = act.tile([2, 128], f32)
            pe_transpose(lsumT_sb[:], lsum_sb[:], 128, 2)
            lsumT_dram = dram.tile([2, 128], f32)
            gsumT_dram = dram.tile([2, 128], f32, addr_space="Shared")
            nc.sync.dma_start(lsumT_dram[:], lsumT_sb[:])
            nc.gpsimd.collective_compute(
                "AllReduce", mybir.AluOpType.add,
                replica_groups=[list(range(NCORES))],
                ins=[lsumT_dram[:]], outs=[gsumT_dram[:]])
            gsumT_sb = act.tile([2, 128], f32)
            nc.sync.dma_start(gsumT_sb[:], gsumT_dram[:])
            loggT_sb = act.tile([2, 128], f32)
            nc.scalar.activation(loggT_sb[:], gsumT_sb[:],
                                 mybir.ActivationFunctionType.Ln)
            logg_sb = act.tile([128, 2], f32)
            pe_transpose(logg_sb[:], loggT_sb[:], 2, 128)

            # ---- logp = logits - log(gsum); write out chunked ----
            for m in range(2):
                for n in range(NV):
                    lp = act.tile([128, VC], f32, tag="lpstage", name="lp",
                                  bufs=2)
                    nc.vector.tensor_scalar_sub(
                        lp[:], logits[m][:, VC * n:VC * (n + 1)],
                        logg_sb[:, m:m + 1])
                    nc.scalar.dma_start(
                        logp_ext[128 * m:128 * (m + 1), VC * n:VC * (n + 1)],
                        lp[:])

    nc.compile()
    return nc


def stage_inputs(input_ids, hidden, encoder_outputs, emb_table,
                 attn_w, attn_b, comb_w, comb_b,
                 w_ih, b_ih, w_hh, b_hh, out_w, out_b, use_bf16=USE_BF16):
    if use_bf16:
        import ml_dtypes
        np_dt = ml_dtypes.bfloat16
    else:
        np_dt = np.float32
    f32 = np.float32

    def cvt(x, dtype):
        return np.ascontiguousarray(np.asarray(x), dtype=dtype)

    ids = np.asarray(input_ids).astype(np.int32).reshape(1, B)
    h0 = cvt(hidden, f32).reshape(B, H)
    enc = cvt(encoder_outputs, np_dt)
    emb = cvt(emb_table, f32)
    # attn_w.T (2H, L) packed into (128, 16*L): [p, L*k + l] = attn_w.T[128k+p, l]
    awT = np.asarray(attn_w).T.astype(np_dt)  # (2H, L)
    awT_packed = np.ascontiguousarray(
        awT.reshape(2 * KH, 128, L).transpose(1, 0, 2).reshape(128, 2 * KH * L))
    ab = cvt(attn_b, np_dt).reshape(1, L)
    cwT = cvt(np.asarray(comb_w).T, np_dt)
    cb = cvt(comb_b, np_dt).reshape(1, H)
    wihT = cvt(np.asarray(w_ih).T, np_dt)
    bih = cvt(b_ih, np_dt).reshape(1, 3 * H)
    whhT = cvt(np.asarray(w_hh).T, np_dt)
    bhh = cvt(b_hh, np_dt).reshape(1, 3 * H)
    owT_full = np.asarray(out_w).T  # (H, V)
    ob = cvt(out_b, np_dt).reshape(1, V)
    ident = np.eye(128, dtype=f32)

    in_maps = []
    for j in range(NCORES):
        bsl = slice(BS * j, BS * (j + 1))
        vsl = slice(VS * j, VS * (j + 1))
        h0_j = np.ascontiguousarray(h0[bsl])
        # packed transposed hidden: h0T[p, BS*k + b] = h0_j[b, 128k + p]
        h0T_j = np.ascontiguousarray(
            h0_j.T.reshape(KH, 128, BS).transpose(1, 0, 2).reshape(128, KH * BS),
            dtype=np_dt)
        # encoder shard packed as (L, BS*H): [l, 1024*b + h]
        enc_j = np.ascontiguousarray(
            enc[bsl].transpose(1, 0, 2).reshape(L, BS * H))
        in_maps.append({
            "ids": np.ascontiguousarray(ids[:, bsl]),
            "emb": emb,
            "h0t": h0T_j,
            "h0": h0_j,
            "enc": enc_j,
            "attn_wt": awT_packed,
            "attn_b": ab,
            "comb_wt": cwT,
            "comb_b": cb,
            "w_iht": wihT,
            "b_ih": bih,
            "w_hht": whhT,
            "b_hh": bhh,
            "out_wt": np.ascontiguousarray(owT_full[:, vsl], dtype=np_dt),
            "out_b": np.ascontiguousarray(ob[:, vsl]),
            "ident": ident,
        })
    return in_maps


def run(inputs, trace=False, trace_cores=None, use_bf16=USE_BF16):
    nc = build_graph(use_bf16)
    in_maps = stage_inputs(**inputs, use_bf16=use_bf16)
    res = run_bass_kernel_spmd(
        nc, in_maps, core_ids=list(range(NCORES)),
        trace=trace, trace_cores=trace_cores)
    r = res.results
    logp = np.concatenate([r[j]["logp"] for j in range(NCORES)], axis=1)
    hnew = np.concatenate([r[j]["hnew"] for j in range(NCORES)], axis=0)[None]
    aw = np.concatenate([r[j]["awout"] for j in range(NCORES)], axis=0)
    return (logp, hnew, aw), res


def kernel(**inputs):
    outs, _ = run(inputs, trace=False)
    return outs


# revision 9
# speedup vs baseline: 1.6154x; 1.0225x over previous
"""Trainium2 8-core SPMD Bass kernel for a single AttnDecoderRNN step.

Reference computation (per step):
  embedded = emb_table[input_ids]                       (1, B, H)
  attn_w'  = softmax([emb; h] @ attn_w.T + attn_b)      (B, L)
  attn_app = einsum('bl,blh->bh', attn_w', enc)         (B, H)
  x        = relu([emb; attn_app] @ comb_w.T + comb_b)  (B, H)
  GRU step (r, z, n gates)  -> h_new                    (B, H)
  logp     = log_softmax(h_new @ out_w.T + out_b)       (B, V)

Sharding: data-parallel over batch B (front: embedding/attention/GRU),
tensor-parallel over vocab V for the output projection with a sharded
log_softmax (AllGather of h_new, AllReduce of sum-exp).

Layout convention on device: matmuls compute out[M,N] = lhsT[K,M].T @ rhs[K,N].
Activations are kept in [batch, feature] layout; the lhsT (stationary) operand
is the transposed activation [feature, batch], produced either by host-side
pre-transposition (inputs) or on-device PE transposes (intermediates).
Weight streams use wide per-k-chunk tiles (one DMA per 128-row chunk) with
k-outer / n-inner matmul loops accumulating into per-n PSUM banks.
"""

import sys

import numpy as np

sys.path.insert(0, "/opt/trn_rl_repo")

from concourse import bass, bacc, mybir, tile  # noqa: E402
from concourse.bass_utils import run_bass_kernel_spmd  # noqa: E402

H, V, L, B = 1024, 32000, 128, 256
NCORES = 8
BS = B // NCORES          # 32 batch rows per core
VS = V // NCORES          # 4000 vocab rows per core
KH = H // 128             # 8 hidden k-chunks
NV = 8                    # vocab n-chunks per core
VC = VS // NV             # 500 columns per vocab chunk
GRU_NC = 3 * H // 512     # 6 n-chunks for the 3H GRU gate matmuls
EB = 2                    # encoder batch rows per DMA tile

USE_BF16 = True


def build_graph(use_bf16=USE_BF16):
    f32 = mybir.dt.float32
    i32 = mybir.dt.int32
    dt = mybir.dt.bfloat16 if use_bf16 else f32

    nc = bacc.Bacc("TRN2", target_bir_lowering=False, debug=False,
                   num_devices=NCORES)

    # ---- kernel I/O ----
    ids_ext = nc.dram_tensor("ids", [1, BS], i32, kind="ExternalInput")
    emb_ext = nc.dram_tensor("emb", [V, H], f32, kind="ExternalInput")
    h0T_ext = nc.dram_tensor("h0t", [128, KH * BS], dt, kind="ExternalInput")
    h0_ext = nc.dram_tensor("h0", [BS, H], f32, kind="ExternalInput")
    # encoder outputs packed as enc[l, B_s*1024 + h] = enc_orig[b, l, h]
    enc_ext = nc.dram_tensor("enc", [L, BS * H], dt, kind="ExternalInput")
    # attn_w.T packed as awT[p, 128*k + l] = attn_w[l, 128*k + p]
    awT_ext = nc.dram_tensor("attn_wt", [128, 2 * KH * L], dt,
                             kind="ExternalInput")
    ab_ext = nc.dram_tensor("attn_b", [1, L], dt, kind="ExternalInput")
    cwT_ext = nc.dram_tensor("comb_wt", [2 * H, H], dt, kind="ExternalInput")
    cb_ext = nc.dram_tensor("comb_b", [1, H], dt, kind="ExternalInput")
    wihT_ext = nc.dram_tensor("w_iht", [H, 3 * H], dt, kind="ExternalInput")
    bih_ext = nc.dram_tensor("b_ih", [1, 3 * H], dt, kind="ExternalInput")
    whhT_ext = nc.dram_tensor("w_hht", [H, 3 * H], dt, kind="ExternalInput")
    bhh_ext = nc.dram_tensor("b_hh", [1, 3 * H], dt, kind="ExternalInput")
    owT_ext = nc.dram_tensor("out_wt", [H, VS], dt, kind="ExternalInput")
    ob_ext = nc.dram_tensor("out_b", [1, VS], dt, kind="ExternalInput")
    ident_ext = nc.dram_tensor("ident", [128, 128], f32, kind="ExternalInput")

    logp_ext = nc.dram_tensor("logp", [B, VS], f32, kind="ExternalOutput")
    hnew_ext = nc.dram_tensor("hnew", [BS, H], f32, kind="ExternalOutput")
    awout_ext = nc.dram_tensor("awout", [BS, L], f32, kind="ExternalOutput")

    with tile.TileContext(nc) as tc:
        from contextlib import ExitStack
        ctx = ExitStack()
        with ctx:
            const = ctx.enter_context(tc.tile_pool(name="const", bufs=1))
            act = ctx.enter_context(tc.tile_pool(name="act", bufs=1))
            wt = ctx.enter_context(tc.tile_pool(name="wt", bufs=2))
            dram = ctx.enter_context(tc.tile_pool(name="dram", bufs=1,
                                                  space="DRAM"))
            pp = ctx.enter_context(tc.tile_pool(name="pp", bufs=8,
                                                space="PSUM"))

            def psum(name):
                return pp.tile([BS, 512], f32, name=name, tag="ps")

            def psum128(name):
                return pp.tile([128, 512], f32, name=name, tag="ps")

            # ---- constants / small inputs ----
            ident = const.tile([128, 128], f32)
            nc.sync.dma_start(ident[:], ident_ext[:])
            def bias_bcast(ext, n, name):
                bt = const.tile([128, n], dt, name=name)
                nc.scalar.dma_start(bt[:], ext[0:1, :].partition_broadcast(128))
                return bt

            ab_bc = bias_bcast(ab_ext, L, "ab_bc")
            cb_bc = bias_bcast(cb_ext, H, "cb_bc")
            bih_bc = bias_bcast(bih_ext, 3 * H, "bih_bc")
            bhh_bc = bias_bcast(bhh_ext, 3 * H, "bhh_bc")
            ob_bc = bias_bcast(ob_ext, VS, "ob_bc")

            h0T_sb = const.tile([128, KH * BS], dt)
            nc.sync.dma_start(h0T_sb[:], h0T_ext[:])
            h0_sb = const.tile([BS, H], f32)
            nc.sync.dma_start(h0_sb[:], h0_ext[:])

            ident_bf = const.tile([128, 128], mybir.dt.bfloat16)
            nc.vector.tensor_copy(ident_bf[:], ident[:])

            def pe_transpose(dst_ap, src_ap, p, m, bf=False):
                """dst[m,p] (SBUF, any dtype) = src[p,m].T via PE; p=src parts."""
                if bf:
                    pt = pp.tile([128, 128], mybir.dt.bfloat16, name="ptb",
                                 tag="ps")
                    idn = ident_bf
                else:
                    pt = psum128("pt")
                    idn = ident
                nc.tensor.transpose(pt[:m, :p], src_ap, idn[:p, :p])
                nc.vector.tensor_copy(dst_ap, pt[:m, :p])

            # ---- GRU gh = h0 @ w_hh.T + b_hh (independent of the gather,
            #      runs first so PE has dense work while the gather runs) ----
            gh_sb = act.tile([BS, 3 * H], dt)
            pgs = [psum(f"pgh{n}") for n in range(GRU_NC)]
            for k in range(KH):
                gwt = wt.tile([128, 3 * H], dt, name="gwt", tag="w3072",
                              bufs=3)
                nc.sync.dma_start(gwt[:], whhT_ext[128 * k:128 * (k + 1), :])
                for n in range(GRU_NC):
                    nc.tensor.matmul(pgs[n][:], h0T_sb[:, BS * k:BS * (k + 1)],
                                     gwt[:, 512 * n:512 * (n + 1)],
                                     start=(k == 0), stop=(k == KH - 1),
                                     skip_group_check=True)
            for n in range(GRU_NC):
                nc.vector.tensor_add(gh_sb[:, 512 * n:512 * (n + 1)],
                                     pgs[n][:],
                                     bhh_bc[:BS, 512 * n:512 * (n + 1)])

            # ---- embedding gather (data-dependent row DMAs, gpsimd) ----
            ids_sb = const.tile([1, BS], i32)
            emb_rows = act.tile([BS, H], f32)
            g_engines = [nc.gpsimd, nc.scalar, nc.sync]
            g_etypes = [mybir.EngineType.Pool, mybir.EngineType.Activation,
                        mybir.EngineType.SP]
            g_sems = [nc.alloc_semaphore(f"gather_sem{e}") for e in range(3)]
            g_cnt = [0, 0, 0]
            with tc.tile_critical():
                nc.gpsimd.dma_start(ids_sb[:], ids_ext[:]).then_inc(g_sems[0],
                                                                    16)
                for eng in g_engines:
                    eng.wait_ge(g_sems[0], 16)
                for i in range(BS):
                    ei = i % 3
                    eng = g_engines[ei]
                    reg = nc.alloc_register(g_etypes[ei], f"embidx{i}")
                    eng.load(reg, ids_sb[0:1, i:i + 1])
                    sv = nc.snap(reg, donate=True, min_val=0, max_val=V - 1)
                    eng.dma_start(
                        out=emb_rows[i:i + 1, :],
                        in_=emb_ext[bass.ds(sv, 1), :],
                    ).then_inc(g_sems[ei], 16)
                    g_cnt[ei] += 1
                nc.gpsimd.wait_ge(g_sems[0], 16 * (1 + g_cnt[0]))
                nc.scalar.wait_ge(g_sems[1], 16 * g_cnt[1])
                nc.sync.wait_ge(g_sems[2], 16 * g_cnt[2])

            # ---- embT ----
            embT = act.tile([128, KH * BS], dt)
            for k in range(KH):
                pe_transpose(embT[:, BS * k:BS * (k + 1)],
                             emb_rows[:, 128 * k:128 * (k + 1)], BS, 128)

            # ---- attn_logits = [emb, h0] @ attn_w.T + attn_b ----
            awt = wt.tile([128, 2 * KH * L], dt, name="awt", tag="wattn",
                          bufs=1)
            nc.sync.dma_start(awt[:], awT_ext[:])
            pa = psum("pa")
            for k in range(2 * KH):
                lhsT = (embT[:, BS * k:BS * (k + 1)] if k < KH
                        else h0T_sb[:, BS * (k - KH):BS * (k - KH + 1)])
                nc.tensor.matmul(pa[:, :L], lhsT,
                                 awt[:, L * k:L * (k + 1)],
                                 start=(k == 0), stop=(k == 2 * KH - 1))

            # ---- softmax over L -> aw; awT; block-diag bd ----
            aw_pre = act.tile([BS, L], f32)
            nc.vector.tensor_add(aw_pre[:], pa[:, :L], ab_bc[:BS, :])
            nmx = act.tile([BS, 1], f32)
            nc.vector.tensor_reduce(nmx[:], aw_pre[:], mybir.AxisListType.X,
                                    mybir.AluOpType.max, negate=True)
            aw_exp = act.tile([BS, L], f32)
            se = act.tile([BS, 1], f32)
            nc.scalar.activation(aw_exp[:], aw_pre[:],
                                 mybir.ActivationFunctionType.Exp,
                                 bias=nmx[:], scale=1.0, accum_out=se[:])
            rse = act.tile([BS, 1], f32)
            nc.vector.reciprocal(rse[:], se[:])
            aw_sb = act.tile([BS, L], f32)
            nc.vector.tensor_scalar_mul(aw_sb[:], aw_exp[:], rse[:])
            nc.scalar.dma_start(awout_ext[:], aw_sb[:])

            awT_sb = act.tile([128, BS], dt)
            pe_transpose(awT_sb[:], aw_sb[:], BS, 128)
            # block-diagonal lhsT: bd[:, 32*b + b] = awT[:, b]
            bd = act.tile([128, BS * BS], dt)
            nc.vector.memset(bd[:], 0.0)
            for b in range(BS):
                nc.vector.tensor_copy(bd[:, 33 * b:33 * b + 1],
                                      awT_sb[:, b:b + 1])

            # ---- attn_applied[b,:] = aw[b,:] @ enc[b] ----
            patt = [psum("patt0"), psum("patt1")]
            for g in range(BS // EB):
                et = wt.tile([128, EB * H], dt, name="enc", tag="enc", bufs=4)
                nc.gpsimd.dma_start(et[:], enc_ext[:, EB * H * g:
                                                 EB * H * (g + 1)])
                for bb in range(EB):
                    b = EB * g + bb
                    for c in range(2):
                        nc.tensor.matmul(
                            patt[c][:],
                            bd[:, BS * b:BS * (b + 1)],
                            et[:, H * bb + 512 * c:H * bb + 512 * (c + 1)],
                            start=(b == 0), stop=(b == BS - 1),
                            skip_group_check=True)
            att_sb = act.tile([BS, H], dt)
            for c in range(2):
                nc.vector.tensor_copy(att_sb[:, 512 * c:512 * (c + 1)],
                                      patt[c][:])

            # ---- attT ----
            attT = act.tile([128, KH * BS], dt)
            for k in range(KH):
                pe_transpose(attT[:, BS * k:BS * (k + 1)],
                             att_sb[:, 128 * k:128 * (k + 1)], BS, 128,
                             bf=use_bf16)

            # ---- x = relu([emb, att] @ comb_w.T + comb_b) ----
            x_sb = act.tile([BS, H], dt)
            pxs = [psum("px0"), psum("px1")]
            for k in range(2 * KH):
                cwt = wt.tile([128, H], dt, name="cwt", tag="w1024", bufs=4)
                nc.sync.dma_start(cwt[:], cwT_ext[128 * k:128 * (k + 1), :])
                lhsT = (embT[:, BS * k:BS * (k + 1)] if k < KH
                        else attT[:, BS * (k - KH):BS * (k - KH + 1)])
                for n in range(2):
                    nc.tensor.matmul(pxs[n][:], lhsT,
                                     cwt[:, 512 * n:512 * (n + 1)],
                                     start=(k == 0), stop=(k == 2 * KH - 1),
                                     skip_group_check=True)
            for n in range(2):
                nc.vector.tensor_add(x_sb[:, 512 * n:512 * (n + 1)],
                                     pxs[n][:],
                                     cb_bc[:BS, 512 * n:512 * (n + 1)])
                nc.vector.tensor_scalar_max(x_sb[:, 512 * n:512 * (n + 1)],
                                            x_sb[:, 512 * n:512 * (n + 1)],
                                            0.0)

            # ---- xT ----
            xT = act.tile([128, KH * BS], dt)
            for k in range(KH):
                pe_transpose(xT[:, BS * k:BS * (k + 1)],
                             x_sb[:, 128 * k:128 * (k + 1)], BS, 128,
                             bf=use_bf16)

            # ---- gx = x @ w_ih.T + b_ih ----
            gx_sb = act.tile([BS, 3 * H], dt)
            pgx = [psum(f"pgx{n}") for n in range(GRU_NC)]
            for k in range(KH):
                gwt2 = wt.tile([128, 3 * H], dt, name="gwt2", tag="w3072",
                               bufs=3)
                nc.sync.dma_start(gwt2[:], wihT_ext[128 * k:128 * (k + 1), :])
                for n in range(GRU_NC):
                    nc.tensor.matmul(pgx[n][:], xT[:, BS * k:BS * (k + 1)],
                                     gwt2[:, 512 * n:512 * (n + 1)],
                                     start=(k == 0), stop=(k == KH - 1),
                                     skip_group_check=True)
            for n in range(GRU_NC):
                nc.vector.tensor_add(gx_sb[:, 512 * n:512 * (n + 1)],
                                     pgx[n][:],
                                     bih_bc[:BS, 512 * n:512 * (n + 1)])

            # ---- GRU elementwise -> h_new ----
            r_sb = act.tile([BS, H], dt)
            z_sb = act.tile([BS, H], dt)
            n_sb = act.tile([BS, H], dt)
            hnew_sb = act.tile([BS, H], f32)
            hnew_bf = act.tile([BS, H], dt)
            for c in range(2):
                cs = slice(512 * c, 512 * (c + 1))
                t1 = act.tile([BS, 512], dt, tag="gtmp", bufs=4, name="t1")
                nc.vector.tensor_add(t1[:], gx_sb[:, cs], gh_sb[:, cs])
                nc.scalar.activation(r_sb[:, cs], t1[:],
                                     mybir.ActivationFunctionType.Sigmoid)
                t2 = act.tile([BS, 512], dt, tag="gtmp", bufs=4, name="t2")
                nc.vector.tensor_add(t2[:], gx_sb[:, H + 512 * c:H + 512 * (c + 1)],
                                     gh_sb[:, H + 512 * c:H + 512 * (c + 1)])
                nc.scalar.activation(z_sb[:, cs], t2[:],
                                     mybir.ActivationFunctionType.Sigmoid)
                t3 = act.tile([BS, 512], dt, tag="gtmp", bufs=4, name="t3")
                nc.vector.tensor_mul(t3[:], r_sb[:, cs],
                                     gh_sb[:, 2 * H + 512 * c:2 * H + 512 * (c + 1)])
                nc.vector.tensor_add(t3[:], t3[:],
                                     gx_sb[:, 2 * H + 512 * c:2 * H + 512 * (c + 1)])
                nc.scalar.activation(n_sb[:, cs], t3[:],
                                     mybir.ActivationFunctionType.Tanh)
                t4 = act.tile([BS, 512], f32, tag="gtmp2", bufs=2, name="t4")
                nc.vector.tensor_sub(t4[:], h0_sb[:, cs], n_sb[:, cs])
                nc.vector.tensor_mul(t4[:], z_sb[:, cs], t4[:])
                nc.vector.tensor_add(hnew_sb[:, cs], n_sb[:, cs], t4[:])
                nc.vector.tensor_copy(hnew_bf[:, cs], hnew_sb[:, cs])
            nc.scalar.dma_start(hnew_ext[:], hnew_sb[:])

            # ---- AllGather h_new across cores (bf16 payload) ----
            hnew_bounce = dram.tile([BS, H], dt)
            h_all_dram = dram.tile([B, H], dt, addr_space="Shared")
            nc.sync.dma_start(hnew_bounce[:], hnew_bf[:])
            nc.gpsimd.collective_compute(
                "AllGather", mybir.AluOpType.bypass,
                replica_groups=[list(range(NCORES))],
                ins=[hnew_bounce[:]], outs=[h_all_dram[:]])
            h_allT = []
            for m in range(2):
                ha = act.tile([128, H], dt, tag="h_all", name=f"h_all{m}")
                nc.gpsimd.dma_start(ha[:], h_all_dram[128 * m:128 * (m + 1), :])
                for k in range(KH):
                    if m == 0:
                        h_allT.append(act.tile([128, 2 * 128], dt,
                                               name=f"h_allT{k}",
                                               tag=f"h_allT{k}"))
                    pe_transpose(h_allT[k][:, 128 * m:128 * (m + 1)],
                                 ha[:, 128 * k:128 * (k + 1)], 128, 128,
                                 bf=use_bf16)

            # ---- logits = h_all @ out_w.T + out_b; exp partial sums ----
            # vocab in 2 halves of 2000; k-outer so each out_w k-chunk is
            # DMA'd once and serves both batch halves m.
            HNV = NV // 2                  # 4 n-chunks per half
            logits = [act.tile([128, VS], dt, tag=f"logits{m}",
                               name=f"logits{m}") for m in range(2)]
            lsums = act.tile([128, 2 * NV], f32)
            for half in range(2):
                pls = [pp.tile([128, VC], f32, name=f"pl{m}_{n}", tag="ps")
                       for m in range(2) for n in range(HNV)]
                for k in range(KH):
                    owt = wt.tile([128, VS // 2], dt, name="owt", tag="w2000",
                                  bufs=6)
                    nc.scalar.dma_start(
                        owt[:], owT_ext[128 * k:128 * (k + 1),
                                        (VS // 2) * half:(VS // 2) * (half + 1)])
                    for m in range(2):
                        for n in range(HNV):
                            nc.tensor.matmul(
                                pls[HNV * m + n][:],
                                h_allT[k][:, 128 * m:128 * (m + 1)],
                                owt[:, VC * n:VC * (n + 1)],
                                start=(k == 0), stop=(k == KH - 1),
                                skip_group_check=True)
                for m in range(2):
                    for n in range(HNV):
                        nv = HNV * half + n
                        nc.vector.tensor_add(
                            logits[m][:, VC * nv:VC * (nv + 1)],
                            pls[HNV * m + n][:],
                            ob_bc[:, VC * nv:VC * (nv + 1)])
                        esc = act.tile([128, VC], f32, tag="lpstage",
                                       name="esc", bufs=2)
                        nc.scalar.activation(
                            esc[:], logits[m][:, VC * nv:VC * (nv + 1)],
                            mybir.ActivationFunctionType.Exp,
                            accum_out=lsums[:, NV * m + nv:NV * m + nv + 1])

            # ---- global log-sum-exp: one AllReduce per vocab half (the
            #      first overlaps the second half's matmuls) ----
            gsumT_sbs = []
            for half in range(2):
                lsum_sb = act.tile([128, 2], f32, name=f"lsum{half}",
                                   tag=f"lsum{half}")
                for m in range(2):
                    nc.vector.tensor_reduce(
                        lsum_sb[:, m:m + 1],
                        lsums[:, NV * m + HNV * half:NV * m + HNV * (half + 1)],
                        mybir.AxisListType.X, mybir.AluOpType.add)
                lsumT_sb = act.tile([2, 128], f32, name=f"lsumT{half}",
                                    tag=f"lsumT{half}")
                pe_transpose(lsumT_sb[:], lsum_sb[:], 128, 2)
                lsumT_dram = dram.tile([2, 128], f32, name=f"lsumd{half}")
                gsumT_dram = dram.tile([2, 128], f32, addr_space="Shared",
                                       name=f"gsumd{half}")
                nc.sync.dma_start(lsumT_dram[:], lsumT_sb[:])
                nc.gpsimd.collective_compute(
                    "AllReduce", mybir.AluOpType.add,
                    replica_groups=[list(range(NCORES))],
                    ins=[lsumT_dram[:]], outs=[gsumT_dram[:]])
                gsumT_sb = act.tile([2, 128], f32, name=f"gsumT{half}",
                                    tag=f"gsumT{half}")
                nc.sync.dma_start(gsumT_sb[:], gsumT_dram[:])
                gsumT_sbs.append(gsumT_sb)
            loggT_sb = act.tile([2, 128], f32)
            nc.vector.tensor_add(loggT_sb[:], gsumT_sbs[0][:], gsumT_sbs[1][:])
            nc.scalar.activation(loggT_sb[:], loggT_sb[:],
                                 mybir.ActivationFunctionType.Ln)
            logg_sb = act.tile([128, 2], f32)
            pe_transpose(logg_sb[:], loggT_sb[:], 2, 128)

            # ---- logp = logits - log(gsum); write out chunked ----
            for m in range(2):
                for n in range(NV):
                    lp = act.tile([128, VC], f32, tag="lpstage", name="lp",
                                  bufs=2)
                    nc.vector.tensor_scalar_sub(
                        lp[:], logits[m][:, VC * n:VC * (n + 1)],
                        logg_sb[:, m:m + 1])
                    nc.scalar.dma_start(
                        logp_ext[128 * m:128 * (m + 1), VC * n:VC * (n + 1)],
                        lp[:])

    nc.compile()
    return nc


def stage_inputs(input_ids, hidden, encoder_outputs, emb_table,
                 attn_w, attn_b, comb_w, comb_b,
                 w_ih, b_ih, w_hh, b_hh, out_w, out_b, use_bf16=USE_BF16):
    if use_bf16:
        import ml_dtypes
        np_dt = ml_dtypes.bfloat16
    else:
        np_dt = np.float32
    f32 = np.float32

    def cvt(x, dtype):
        return np.ascontiguousarray(np.asarray(x), dtype=dtype)

    ids = np.asarray(input_ids).astype(np.int32).reshape(1, B)
    h0 = cvt(hidden, f32).reshape(B, H)
    enc = cvt(encoder_outputs, np_dt)
    emb = cvt(emb_table, f32)
    # attn_w.T (2H, L) packed into (128, 16*L): [p, L*k + l] = attn_w.T[128k+p, l]
    awT = np.asarray(attn_w).T.astype(np_dt)  # (2H, L)
    awT_packed = np.ascontiguousarray(
        awT.reshape(2 * KH, 128, L).transpose(1, 0, 2).reshape(128, 2 * KH * L))
    ab = cvt(attn_b, np_dt).reshape(1, L)
    cwT = cvt(np.asarray(comb_w).T, np_dt)
    cb = cvt(comb_b, np_dt).reshape(1, H)
    wihT = cvt(np.asarray(w_ih).T, np_dt)
    bih = cvt(b_ih, np_dt).reshape(1, 3 * H)
    whhT = cvt(np.asarray(w_hh).T, np_dt)
    bhh = cvt(b_hh, np_dt).reshape(1, 3 * H)
    owT_full = np.asarray(out_w).T  # (H, V)
    ob = cvt(out_b, np_dt).reshape(1, V)
    ident = np.eye(128, dtype=f32)

    in_maps = []
    for j in range(NCORES):
        bsl = slice(BS * j, BS * (j + 1))
        vsl = slice(VS * j, VS * (j + 1))
        h0_j = np.ascontiguousarray(h0[bsl])
        # packed transposed hidden: h0T[p, BS*k + b] = h0_j[b, 128k + p]
        h0T_j = np.ascontiguousarray(
            h0_j.T.reshape(KH, 128, BS).transpose(1, 0, 2).reshape(128, KH * BS),
            dtype=np_dt)
        # encoder shard packed as (L, BS*H): [l, 1024*b + h]
        enc_j = np.ascontiguousarray(
            enc[bsl].transpose(1, 0, 2).reshape(L, BS * H))
        in_maps.append({
            "ids": np.ascontiguousarray(ids[:, bsl]),
            "emb": emb,
            "h0t": h0T_j,
            "h0": h0_j,
            "enc": enc_j,
            "attn_wt": awT_packed,
            "attn_b": ab,
            "comb_wt": cwT,
            "comb_b": cb,
            "w_iht": wihT,
            "b_ih": bih,
            "w_hht": whhT,
            "b_hh": bhh,
            "out_wt": np.ascontiguousarray(owT_full[:, vsl], dtype=np_dt),
            "out_b": np.ascontiguousarray(ob[:, vsl]),
            "ident": ident,
        })
    return in_maps


def run(inputs, trace=False, trace_cores=None, use_bf16=USE_BF16):
    nc = build_graph(use_bf16)
    in_maps = stage_inputs(**inputs, use_bf16=use_bf16)
    res = run_bass_kernel_spmd(
        nc, in_maps, core_ids=list(range(NCORES)),
        trace=trace, trace_cores=trace_cores)
    r = res.results
    logp = np.concatenate([r[j]["logp"] for j in range(NCORES)], axis=1)
    hnew = np.concatenate([r[j]["hnew"] for j in range(NCORES)], axis=0)[None]
    aw = np.concatenate([r[j]["awout"] for j in range(NCORES)], axis=0)
    return (logp, hnew, aw), res


def kernel(**inputs):
    outs, _ = run(inputs, trace=False)
    return outs


# revision 11
# speedup vs baseline: 1.7701x; 1.0958x over previous
"""Trainium2 8-core SPMD Bass kernel for a single AttnDecoderRNN step.

Reference computation (per step):
  embedded = emb_table[input_ids]                       (1, B, H)
  attn_w'  = softmax([emb; h] @ attn_w.T + attn_b)      (B, L)
  attn_app = einsum('bl,blh->bh', attn_w', enc)         (B, H)
  x        = relu([emb; attn_app] @ comb_w.T + comb_b)  (B, H)
  GRU step (r, z, n gates)  -> h_new                    (B, H)
  logp     = log_softmax(h_new @ out_w.T + out_b)       (B, V)

Sharding: data-parallel over batch B (front: embedding/attention/GRU),
tensor-parallel over vocab V for the output projection with a sharded
log_softmax (AllGather of h_new, AllReduce of sum-exp).

Layout convention on device: matmuls compute out[M,N] = lhsT[K,M].T @ rhs[K,N].
Activations are kept in [batch, feature] layout; the lhsT (stationary) operand
is the transposed activation [feature, batch], produced either by host-side
pre-transposition (inputs) or on-device PE transposes (intermediates).
Weight streams use wide per-k-chunk tiles (one DMA per 128-row chunk) with
k-outer / n-inner matmul loops accumulating into per-n PSUM banks.
"""

import sys

import numpy as np

sys.path.insert(0, "/opt/trn_rl_repo")

from concourse import bass, bacc, mybir, tile  # noqa: E402
from concourse.bass_utils import run_bass_kernel_spmd  # noqa: E402

H, V, L, B = 1024, 32000, 128, 256
NCORES = 8
BS = B // NCORES          # 32 batch rows per core
VS = V // NCORES          # 4000 vocab rows per core
KH = H // 128             # 8 hidden k-chunks
NV = 8                    # vocab n-chunks per core
VC = VS // NV             # 500 columns per vocab chunk
GRU_NC = 3 * H // 512     # 6 n-chunks for the 3H GRU gate matmuls
EB = 2                    # encoder batch rows per DMA tile

USE_BF16 = True


def build_graph(use_bf16=USE_BF16):
    f32 = mybir.dt.float32
    i32 = mybir.dt.int32
    dt = mybir.dt.bfloat16 if use_bf16 else f32

    nc = bacc.Bacc("TRN2", target_bir_lowering=False, debug=False,
                   num_devices=NCORES)

    # ---- kernel I/O ----
    ids_ext = nc.dram_tensor("ids", [1, BS], i32, kind="ExternalInput")
    emb_ext = nc.dram_tensor("emb", [V, H], f32, kind="ExternalInput")
    h0T_ext = nc.dram_tensor("h0t", [128, KH * BS], dt, kind="ExternalInput")
    h0_ext = nc.dram_tensor("h0", [BS, H], f32, kind="ExternalInput")
    # encoder outputs packed as enc[l, B_s*1024 + h] = enc_orig[b, l, h]
    enc_ext = nc.dram_tensor("enc", [L, BS * H], dt, kind="ExternalInput")
    # attn_w.T packed as awT[p, 128*k + l] = attn_w[l, 128*k + p]
    awT_ext = nc.dram_tensor("attn_wt", [128, 2 * KH * L], dt,
                             kind="ExternalInput")
    ab_ext = nc.dram_tensor("attn_b", [1, L], dt, kind="ExternalInput")
    cwT_ext = nc.dram_tensor("comb_wt", [2 * H, H], dt, kind="ExternalInput")
    cb_ext = nc.dram_tensor("comb_b", [1, H], dt, kind="ExternalInput")
    wihT_ext = nc.dram_tensor("w_iht", [H, 3 * H], dt, kind="ExternalInput")
    bih_ext = nc.dram_tensor("b_ih", [1, 3 * H], dt, kind="ExternalInput")
    whhT_ext = nc.dram_tensor("w_hht", [H, 3 * H], dt, kind="ExternalInput")
    bhh_ext = nc.dram_tensor("b_hh", [1, 3 * H], dt, kind="ExternalInput")
    owT_ext = nc.dram_tensor("out_wt", [H, VS], dt, kind="ExternalInput")
    ob_ext = nc.dram_tensor("out_b", [1, VS], dt, kind="ExternalInput")
    ident_ext = nc.dram_tensor("ident", [128, 128], f32, kind="ExternalInput")

    logp_ext = nc.dram_tensor("logp", [B, VS], f32, kind="ExternalOutput")
    hnew_ext = nc.dram_tensor("hnew", [BS, H], f32, kind="ExternalOutput")
    awout_ext = nc.dram_tensor("awout", [BS, L], f32, kind="ExternalOutput")

    with tile.TileContext(nc) as tc:
        from contextlib import ExitStack
        ctx = ExitStack()
        with ctx:
            const = ctx.enter_context(tc.tile_pool(name="const", bufs=1))
            act = ctx.enter_context(tc.tile_pool(name="act", bufs=1))
            wt = ctx.enter_context(tc.tile_pool(name="wt", bufs=2))
            dram = ctx.enter_context(tc.tile_pool(name="dram", bufs=1,
                                                  space="DRAM"))
            pp = ctx.enter_context(tc.tile_pool(name="pp", bufs=8,
                                                space="PSUM"))

            def psum(name):
                return pp.tile([BS, 512], f32, name=name, tag="ps")

            def psum128(name):
                return pp.tile([128, 512], f32, name=name, tag="ps")

            # ---- constants / small inputs ----
            ident = const.tile([128, 128], f32)
            nc.sync.dma_start(ident[:], ident_ext[:])
            def bias_bcast(ext, n, name):
                bt = const.tile([128, n], dt, name=name)
                nc.scalar.dma_start(bt[:], ext[0:1, :].partition_broadcast(128))
                return bt

            ab_bc = bias_bcast(ab_ext, L, "ab_bc")
            cb_bc = bias_bcast(cb_ext, H, "cb_bc")
            bih_bc = bias_bcast(bih_ext, 3 * H, "bih_bc")
            bhh_bc = bias_bcast(bhh_ext, 3 * H, "bhh_bc")
            ob_bc = bias_bcast(ob_ext, VS, "ob_bc")

            h0T_sb = const.tile([128, KH * BS], dt)
            nc.sync.dma_start(h0T_sb[:], h0T_ext[:])
            h0_sb = const.tile([BS, H], f32)
            nc.sync.dma_start(h0_sb[:], h0_ext[:])

            ident_bf = const.tile([128, 128], mybir.dt.bfloat16)
            nc.vector.tensor_copy(ident_bf[:], ident[:])
            bd = const.tile([128, BS * BS], dt)
            nc.vector.memset(bd[:], 0.0)

            def pe_transpose(dst_ap, src_ap, p, m, bf=False):
                """dst[m,p] (SBUF, any dtype) = src[p,m].T via PE; p=src parts."""
                if bf:
                    pt = pp.tile([128, 128], mybir.dt.bfloat16, name="ptb",
                                 tag="ps")
                    idn = ident_bf
                else:
                    pt = psum128("pt")
                    idn = ident
                nc.tensor.transpose(pt[:m, :p], src_ap, idn[:p, :p])
                nc.vector.tensor_copy(dst_ap, pt[:m, :p])

            # ---- embedding gather (data-dependent row DMAs, gpsimd) ----
            ids_sb = const.tile([1, BS], i32)
            emb_rows = act.tile([BS, H], f32)
            g_engines = [nc.gpsimd, nc.scalar]
            g_etypes = [mybir.EngineType.Pool, mybir.EngineType.Activation]
            g_sems = [nc.alloc_semaphore(f"gather_sem{e}") for e in range(3)]
            g_cnt = [0, 0]
            nc.gpsimd.dma_start(ids_sb[:], ids_ext[:]).then_inc(g_sems[2], 16)
            with tc.tile_critical():
                for eng in g_engines:
                    eng.wait_ge(g_sems[2], 16)
                for i in range(BS):
                    ei = i % 2
                    eng = g_engines[ei]
                    reg = nc.alloc_register(g_etypes[ei], f"embidx{i}")
                    eng.load(reg, ids_sb[0:1, i:i + 1])
                    sv = nc.snap(reg, donate=True, min_val=0, max_val=V - 1)
                    eng.dma_start(
                        out=emb_rows[i:i + 1, :],
                        in_=emb_ext[bass.ds(sv, 1), :],
                    ).then_inc(g_sems[ei], 16)
                    g_cnt[ei] += 1
                nc.gpsimd.wait_ge(g_sems[0], 16 * g_cnt[0])
                nc.scalar.wait_ge(g_sems[1], 16 * g_cnt[1])

            # ---- GRU gh = h0 @ w_hh.T + b_hh (independent of the gather,
            #      runs first so PE has dense work while the gather runs) ----
            gh_sb = act.tile([BS, 3 * H], dt)
            pgs = [psum(f"pgh{n}") for n in range(GRU_NC)]
            for k in range(KH):
                gwt = wt.tile([128, 3 * H], dt, name="gwt", tag="w3072",
                              bufs=3)
                nc.sync.dma_start(gwt[:], whhT_ext[128 * k:128 * (k + 1), :])
                for n in range(GRU_NC):
                    nc.tensor.matmul(pgs[n][:], h0T_sb[:, BS * k:BS * (k + 1)],
                                     gwt[:, 512 * n:512 * (n + 1)],
                                     start=(k == 0), stop=(k == KH - 1),
                                     skip_group_check=True)
            for n in range(GRU_NC):
                nc.vector.tensor_add(gh_sb[:, 512 * n:512 * (n + 1)],
                                     pgs[n][:],
                                     bhh_bc[:BS, 512 * n:512 * (n + 1)])

            # ---- embT ----
            embT = act.tile([128, KH * BS], dt)
            for k in range(KH):
                pe_transpose(embT[:, BS * k:BS * (k + 1)],
                             emb_rows[:, 128 * k:128 * (k + 1)], BS, 128)

            # ---- attn_logits = [emb, h0] @ attn_w.T + attn_b ----
            awt = wt.tile([128, 2 * KH * L], dt, name="awt", tag="wattn",
                          bufs=1)
            nc.sync.dma_start(awt[:], awT_ext[:])
            pa = psum("pa")
            for k in range(2 * KH):
                lhsT = (embT[:, BS * k:BS * (k + 1)] if k < KH
                        else h0T_sb[:, BS * (k - KH):BS * (k - KH + 1)])
                nc.tensor.matmul(pa[:, :L], lhsT,
                                 awt[:, L * k:L * (k + 1)],
                                 start=(k == 0), stop=(k == 2 * KH - 1))

            # ---- softmax over L -> aw; awT; block-diag bd ----
            aw_pre = act.tile([BS, L], f32)
            nc.vector.tensor_add(aw_pre[:], pa[:, :L], ab_bc[:BS, :])
            nmx = act.tile([BS, 1], f32)
            nc.vector.tensor_reduce(nmx[:], aw_pre[:], mybir.AxisListType.X,
                                    mybir.AluOpType.max, negate=True)
            aw_exp = act.tile([BS, L], f32)
            se = act.tile([BS, 1], f32)
            nc.scalar.activation(aw_exp[:], aw_pre[:],
                                 mybir.ActivationFunctionType.Exp,
                                 bias=nmx[:], scale=1.0, accum_out=se[:])
            rse = act.tile([BS, 1], f32)
            nc.vector.reciprocal(rse[:], se[:])
            aw_sb = act.tile([BS, L], f32)
            nc.vector.tensor_scalar_mul(aw_sb[:], aw_exp[:], rse[:])
            nc.scalar.dma_start(awout_ext[:], aw_sb[:])

            awT_sb = act.tile([128, BS], dt)
            pe_transpose(awT_sb[:], aw_sb[:], BS, 128)
            # block-diagonal lhsT: bd[:, 32*b + b] = awT[:, b]
            # (bd_z zeroed early, off the critical path)
            nc.vector.tensor_copy(bd[:, 0:BS * BS:BS + 1], awT_sb[:, :])

            # ---- attn_applied[b,:] = aw[b,:] @ enc[b] ----
            patt = [psum("patt0"), psum("patt1")]
            for g in range(BS // EB):
                et = wt.tile([128, EB * H], dt, name="enc", tag="strm", bufs=10)
                nc.gpsimd.dma_start(et[:], enc_ext[:, EB * H * g:
                                                 EB * H * (g + 1)])
                for bb in range(EB):
                    b = EB * g + bb
                    for c in range(2):
                        nc.tensor.matmul(
                            patt[c][:],
                            bd[:, BS * b:BS * (b + 1)],
                            et[:, H * bb + 512 * c:H * bb + 512 * (c + 1)],
                            start=(b == 0), stop=(b == BS - 1),
                            skip_group_check=True)
            att_sb = act.tile([BS, H], dt)
            for c in range(2):
                nc.vector.tensor_copy(att_sb[:, 512 * c:512 * (c + 1)],
                                      patt[c][:])

            # ---- attT ----
            attT = act.tile([128, KH * BS], dt)
            for k in range(KH):
                pe_transpose(attT[:, BS * k:BS * (k + 1)],
                             att_sb[:, 128 * k:128 * (k + 1)], BS, 128,
                             bf=use_bf16)

            # ---- x = relu([emb, att] @ comb_w.T + comb_b) ----
            x_sb = act.tile([BS, H], dt)
            pxs = [psum("px0"), psum("px1")]
            for k in range(2 * KH):
                cwt = wt.tile([128, H], dt, name="cwt", tag="w1024", bufs=4)
                nc.sync.dma_start(cwt[:], cwT_ext[128 * k:128 * (k + 1), :])
                lhsT = (embT[:, BS * k:BS * (k + 1)] if k < KH
                        else attT[:, BS * (k - KH):BS * (k - KH + 1)])
                for n in range(2):
                    nc.tensor.matmul(pxs[n][:], lhsT,
                                     cwt[:, 512 * n:512 * (n + 1)],
                                     start=(k == 0), stop=(k == 2 * KH - 1),
                                     skip_group_check=True)
            for n in range(2):
                nc.vector.tensor_add(x_sb[:, 512 * n:512 * (n + 1)],
                                     pxs[n][:],
                                     cb_bc[:BS, 512 * n:512 * (n + 1)])
                nc.vector.tensor_scalar_max(x_sb[:, 512 * n:512 * (n + 1)],
                                            x_sb[:, 512 * n:512 * (n + 1)],
                                            0.0)

            # ---- xT ----
            xT = act.tile([128, KH * BS], dt)
            for k in range(KH):
                pe_transpose(xT[:, BS * k:BS * (k + 1)],
                             x_sb[:, 128 * k:128 * (k + 1)], BS, 128,
                             bf=use_bf16)

            # ---- gx = x @ w_ih.T + b_ih ----
            gx_sb = act.tile([BS, 3 * H], dt)
            pgx = [psum(f"pgx{n}") for n in range(GRU_NC)]
            for k in range(KH):
                gwt2 = wt.tile([128, 3 * H], dt, name="gwt2", tag="w3072",
                               bufs=3)
                nc.sync.dma_start(gwt2[:], wihT_ext[128 * k:128 * (k + 1), :])
                for n in range(GRU_NC):
                    nc.tensor.matmul(pgx[n][:], xT[:, BS * k:BS * (k + 1)],
                                     gwt2[:, 512 * n:512 * (n + 1)],
                                     start=(k == 0), stop=(k == KH - 1),
                                     skip_group_check=True)
            for n in range(GRU_NC):
                nc.vector.tensor_add(gx_sb[:, 512 * n:512 * (n + 1)],
                                     pgx[n][:],
                                     bih_bc[:BS, 512 * n:512 * (n + 1)])

            # ---- GRU elementwise -> h_new ----
            r_sb = act.tile([BS, H], dt)
            z_sb = act.tile([BS, H], dt)
            n_sb = act.tile([BS, H], dt)
            hnew_sb = act.tile([BS, H], f32)
            hnew_bf = act.tile([BS, H], dt)
            for c in range(2):
                cs = slice(512 * c, 512 * (c + 1))
                t1 = act.tile([BS, 512], dt, tag="gtmp", bufs=4, name="t1")
                nc.vector.tensor_add(t1[:], gx_sb[:, cs], gh_sb[:, cs])
                nc.scalar.activation(r_sb[:, cs], t1[:],
                                     mybir.ActivationFunctionType.Sigmoid)
                t2 = act.tile([BS, 512], dt, tag="gtmp", bufs=4, name="t2")
                nc.vector.tensor_add(t2[:], gx_sb[:, H + 512 * c:H + 512 * (c + 1)],
                                     gh_sb[:, H + 512 * c:H + 512 * (c + 1)])
                nc.scalar.activation(z_sb[:, cs], t2[:],
                                     mybir.ActivationFunctionType.Sigmoid)
                t3 = act.tile([BS, 512], dt, tag="gtmp", bufs=4, name="t3")
                nc.vector.tensor_mul(t3[:], r_sb[:, cs],
                                     gh_sb[:, 2 * H + 512 * c:2 * H + 512 * (c + 1)])
                nc.vector.tensor_add(t3[:], t3[:],
                                     gx_sb[:, 2 * H + 512 * c:2 * H + 512 * (c + 1)])
                nc.scalar.activation(n_sb[:, cs], t3[:],
                                     mybir.ActivationFunctionType.Tanh)
                t4 = act.tile([BS, 512], f32, tag="gtmp2", bufs=2, name="t4")
                nc.vector.tensor_sub(t4[:], h0_sb[:, cs], n_sb[:, cs])
                nc.vector.tensor_mul(t4[:], z_sb[:, cs], t4[:])
                nc.vector.tensor_add(hnew_sb[:, cs], n_sb[:, cs], t4[:])
                nc.vector.tensor_copy(hnew_bf[:, cs], hnew_sb[:, cs])
            nc.scalar.dma_start(hnew_ext[:], hnew_sb[:])

            # ---- AllGather h_new across cores (bf16 payload) ----
            hnew_bounce = dram.tile([BS, H], dt)
            h_all_dram = dram.tile([B, H], dt, addr_space="Shared")
            nc.sync.dma_start(hnew_bounce[:], hnew_bf[:])
            nc.gpsimd.collective_compute(
                "AllGather", mybir.AluOpType.bypass,
                replica_groups=[list(range(NCORES))],
                ins=[hnew_bounce[:]], outs=[h_all_dram[:]])
            h_allT = []
            for m in range(2):
                ha = act.tile([128, H], dt, tag="h_all", name=f"h_all{m}")
                nc.gpsimd.dma_start(ha[:], h_all_dram[128 * m:128 * (m + 1), :])
                for k in range(KH):
                    if m == 0:
                        h_allT.append(act.tile([128, 2 * 128], dt,
                                               name=f"h_allT{k}",
                                               tag=f"h_allT{k}"))
                    pe_transpose(h_allT[k][:, 128 * m:128 * (m + 1)],
                                 ha[:, 128 * k:128 * (k + 1)], 128, 128,
                                 bf=use_bf16)

            # ---- logits = h_all @ out_w.T + out_b; exp partial sums ----
            # vocab in 2 halves of 2000; k-outer so each out_w k-chunk is
            # DMA'd once and serves both batch halves m.
            HNV = NV // 2                  # 4 n-chunks per half
            logits = [act.tile([128, VS], dt, tag=f"logits{m}",
                               name=f"logits{m}") for m in range(2)]
            lsums = act.tile([128, 2 * NV], f32)
            gsumT_sbs = []
            for half in range(2):
                pls = [pp.tile([128, VC], f32, name=f"pl{m}_{n}", tag="ps")
                       for m in range(2) for n in range(HNV)]
                for k in range(KH):
                    owt = wt.tile([128, VS // 2], dt, name="owt", tag="strm",
                                  bufs=10)
                    nc.sync.dma_start(
                        owt[:], owT_ext[128 * k:128 * (k + 1),
                                        (VS // 2) * half:(VS // 2) * (half + 1)])
                    for m in range(2):
                        for n in range(HNV):
                            nc.tensor.matmul(
                                pls[HNV * m + n][:],
                                h_allT[k][:, 128 * m:128 * (m + 1)],
                                owt[:, VC * n:VC * (n + 1)],
                                start=(k == 0), stop=(k == KH - 1),
                                skip_group_check=True)
                for m in range(2):
                    for n in range(HNV):
                        nv = HNV * half + n
                        nc.vector.tensor_add(
                            logits[m][:, VC * nv:VC * (nv + 1)],
                            pls[HNV * m + n][:],
                            ob_bc[:, VC * nv:VC * (nv + 1)])
                        esc = act.tile([128, VC], f32, tag="lpstage",
                                       name="esc", bufs=4)
                        nc.scalar.activation(
                            esc[:], logits[m][:, VC * nv:VC * (nv + 1)],
                            mybir.ActivationFunctionType.Exp,
                            accum_out=lsums[:, NV * m + nv:NV * m + nv + 1])
                # per-half global sum-exp AllReduce; half 0's collective
                # overlaps half 1's matmuls
                lsum_sb = act.tile([128, 2], f32, name=f"lsum{half}",
                                   tag=f"lsum{half}")
                for m in range(2):
                    nc.vector.tensor_reduce(
                        lsum_sb[:, m:m + 1],
                        lsums[:, NV * m + HNV * half:NV * m + HNV * (half + 1)],
                        mybir.AxisListType.X, mybir.AluOpType.add)
                lsumT_sb = act.tile([2, 128], f32, name=f"lsumT{half}",
                                    tag=f"lsumT{half}")
                pe_transpose(lsumT_sb[:], lsum_sb[:], 128, 2)
                lsumT_dram = dram.tile([2, 128], f32, name=f"lsumd{half}")
                gsumT_dram = dram.tile([2, 128], f32, addr_space="Shared",
                                       name=f"gsumd{half}")
                nc.sync.dma_start(lsumT_dram[:], lsumT_sb[:])
                nc.gpsimd.collective_compute(
                    "AllReduce", mybir.AluOpType.add,
                    replica_groups=[list(range(NCORES))],
                    ins=[lsumT_dram[:]], outs=[gsumT_dram[:]])
                gsumT_sb = act.tile([2, 128], f32, name=f"gsumT{half}",
                                    tag=f"gsumT{half}")
                nc.sync.dma_start(gsumT_sb[:], gsumT_dram[:])
                gsumT_sbs.append(gsumT_sb)

            # ---- combine the two per-half partial sums ----
            loggT_sb = act.tile([2, 128], f32)
            nc.vector.tensor_add(loggT_sb[:], gsumT_sbs[0][:], gsumT_sbs[1][:])
            nc.scalar.activation(loggT_sb[:], loggT_sb[:],
                                 mybir.ActivationFunctionType.Ln)
            logg_sb = act.tile([128, 2], f32)
            pe_transpose(logg_sb[:], loggT_sb[:], 2, 128)

            # ---- logp = logits - log(gsum); write out chunked ----
            for m in range(2):
                for n in range(NV):
                    lp = act.tile([128, VC], f32, tag="lpstage", name="lp",
                                  bufs=4)
                    nc.vector.tensor_scalar_sub(
                        lp[:], logits[m][:, VC * n:VC * (n + 1)],
                        logg_sb[:, m:m + 1])
                    (nc.scalar if n % 2 else nc.sync).dma_start(
                        logp_ext[128 * m:128 * (m + 1), VC * n:VC * (n + 1)],
                        lp[:])

    nc.compile()
    return nc


def stage_inputs(input_ids, hidden, encoder_outputs, emb_table,
                 attn_w, attn_b, comb_w, comb_b,
                 w_ih, b_ih, w_hh, b_hh, out_w, out_b, use_bf16=USE_BF16):
    if use_bf16:
        import ml_dtypes
        np_dt = ml_dtypes.bfloat16
    else:
        np_dt = np.float32
    f32 = np.float32

    def cvt(x, dtype):
        return np.ascontiguousarray(np.asarray(x), dtype=dtype)

    ids = np.asarray(input_ids).astype(np.int32).reshape(1, B)
    h0 = cvt(hidden, f32).reshape(B, H)
    enc = cvt(encoder_outputs, np_dt)
    emb = cvt(emb_table, f32)
    # attn_w.T (2H, L) packed into (128, 16*L): [p, L*k + l] = attn_w.T[128k+p, l]
    awT = np.asarray(attn_w).T.astype(np_dt)  # (2H, L)
    awT_packed = np.ascontiguousarray(
        awT.reshape(2 * KH, 128, L).transpose(1, 0, 2).reshape(128, 2 * KH * L))
    ab = cvt(attn_b, np_dt).reshape(1, L)
    cwT = cvt(np.asarray(comb_w).T, np_dt)
    cb = cvt(comb_b, np_dt).reshape(1, H)
    wihT = cvt(np.asarray(w_ih).T, np_dt)
    bih = cvt(b_ih, np_dt).reshape(1, 3 * H)
    whhT = cvt(np.asarray(w_hh).T, np_dt)
    bhh = cvt(b_hh, np_dt).reshape(1, 3 * H)
    owT_full = np.asarray(out_w).T  # (H, V)
    ob = cvt(out_b, np_dt).reshape(1, V)
    ident = np.eye(128, dtype=f32)

    in_maps = []
    for j in range(NCORES):
        bsl = slice(BS * j, BS * (j + 1))
        vsl = slice(VS * j, VS * (j + 1))
        h0_j = np.ascontiguousarray(h0[bsl])
        # packed transposed hidden: h0T[p, BS*k + b] = h0_j[b, 128k + p]
        h0T_j = np.ascontiguousarray(
            h0_j.T.reshape(KH, 128, BS).transpose(1, 0, 2).reshape(128, KH * BS),
            dtype=np_dt)
        # encoder shard packed as (L, BS*H): [l, 1024*b + h]
        enc_j = np.ascontiguousarray(
            enc[bsl].transpose(1, 0, 2).reshape(L, BS * H))
        in_maps.append({
            "ids": np.ascontiguousarray(ids[:, bsl]),
            "emb": emb,
            "h0t": h0T_j,
            "h0": h0_j,
            "enc": enc_j,
            "attn_wt": awT_packed,
            "attn_b": ab,
            "comb_wt": cwT,
            "comb_b": cb,
            "w_iht": wihT,
            "b_ih": bih,
            "w_hht": whhT,
            "b_hh": bhh,
            "out_wt": np.ascontiguousarray(owT_full[:, vsl], dtype=np_dt),
            "out_b": np.ascontiguousarray(ob[:, vsl]),
            "ident": ident,
        })
    return in_maps


def run(inputs, trace=False, trace_cores=None, use_bf16=USE_BF16):
    nc = build_graph(use_bf16)
    in_maps = stage_inputs(**inputs, use_bf16=use_bf16)
    res = run_bass_kernel_spmd(
        nc, in_maps, core_ids=list(range(NCORES)),
        trace=trace, trace_cores=trace_cores)
    r = res.results
    logp = np.concatenate([r[j]["logp"] for j in range(NCORES)], axis=1)
    hnew = np.concatenate([r[j]["hnew"] for j in range(NCORES)], axis=0)[None]
    aw = np.concatenate([r[j]["awout"] for j in range(NCORES)], axis=0)
    return (logp, hnew, aw), res


def kernel(**inputs):
    outs, _ = run(inputs, trace=False)
    return outs


# revision 18
# speedup vs baseline: 2.3221x; 1.3119x over previous
"""Trainium2 8-core SPMD Bass kernel for a single AttnDecoderRNN step.

Reference computation (per step):
  embedded = emb_table[input_ids]                       (1, B, H)
  attn_w'  = softmax([emb; h] @ attn_w.T + attn_b)      (B, L)
  attn_app = einsum('bl,blh->bh', attn_w', enc)         (B, H)
  x        = relu([emb; attn_app] @ comb_w.T + comb_b)  (B, H)
  GRU step (r, z, n gates)  -> h_new                    (B, H)
  logp     = log_softmax(h_new @ out_w.T + out_b)       (B, V)

Sharding: data-parallel over batch B (front: embedding/attention/GRU),
tensor-parallel over vocab V for the output projection with a sharded
log_softmax (AllGather of h_new, AllReduce of sum-exp).

Layout convention on device: matmuls compute out[M,N] = lhsT[K,M].T @ rhs[K,N].
Activations are kept in [batch, feature] layout; the lhsT (stationary) operand
is the transposed activation [feature, batch], produced either by host-side
pre-transposition (inputs) or on-device PE transposes (intermediates).
Weight streams use wide per-k-chunk tiles (one DMA per 128-row chunk) with
k-outer / n-inner matmul loops accumulating into per-n PSUM banks.
"""

import sys

import numpy as np

sys.path.insert(0, "/opt/trn_rl_repo")

from concourse import bass, bacc, mybir, tile  # noqa: E402
from concourse.bass_utils import run_bass_kernel_spmd  # noqa: E402

H, V, L, B = 1024, 32000, 128, 256
NCORES = 8
BS = B // NCORES          # 32 batch rows per core
VS = V // NCORES          # 4000 vocab rows per core
KH = H // 128             # 8 hidden k-chunks
NV = 8                    # vocab n-chunks per core
VC = VS // NV             # 500 columns per vocab chunk
GRU_NC = 3 * H // 512     # 6 n-chunks for the 3H GRU gate matmuls
EB = 2                    # encoder batch rows per DMA tile

USE_BF16 = True


def build_graph(use_bf16=USE_BF16):
    f32 = mybir.dt.float32
    i32 = mybir.dt.int32
    dt = mybir.dt.bfloat16 if use_bf16 else f32

    nc = bacc.Bacc("TRN2", target_bir_lowering=False, debug=False,
                   num_devices=NCORES)

    # ---- kernel I/O ----
    ids_ext = nc.dram_tensor("ids", [1, BS], i32, kind="ExternalInput")
    emb_ext = nc.dram_tensor("emb", [V, H], f32, kind="ExternalInput")
    h0T_ext = nc.dram_tensor("h0t", [128, KH * BS], dt, kind="ExternalInput")
    h0_ext = nc.dram_tensor("h0", [BS, H], f32, kind="ExternalInput")
    # encoder outputs packed as enc[l, B_s*1024 + h] = enc_orig[b, l, h]
    enc_ext = nc.dram_tensor("enc", [L, BS * H], dt, kind="ExternalInput")
    # attn_w.T packed as awT[p, 128*k + l] = attn_w[l, 128*k + p]
    awT_ext = nc.dram_tensor("attn_wt", [128, 2 * KH * L], dt,
                             kind="ExternalInput")
    ab_ext = nc.dram_tensor("attn_b", [1, L], dt, kind="ExternalInput")
    cwT_ext = nc.dram_tensor("comb_wt", [2 * H, H], dt, kind="ExternalInput")
    cb_ext = nc.dram_tensor("comb_b", [1, H], dt, kind="ExternalInput")
    wihT_ext = nc.dram_tensor("w_iht", [H, 3 * H], dt, kind="ExternalInput")
    bih_ext = nc.dram_tensor("b_ih", [1, 3 * H], dt, kind="ExternalInput")
    whhT_ext = nc.dram_tensor("w_hht", [H, 3 * H], dt, kind="ExternalInput")
    bhh_ext = nc.dram_tensor("b_hh", [1, 3 * H], dt, kind="ExternalInput")
    owT_ext = nc.dram_tensor("out_wt", [H, VS], dt, kind="ExternalInput")
    ob_ext = nc.dram_tensor("out_b", [1, VS], dt, kind="ExternalInput")
    ident_ext = nc.dram_tensor("ident", [128, 128], f32, kind="ExternalInput")

    logp_ext = nc.dram_tensor("logp", [B, VS], f32, kind="ExternalOutput")
    hnew_ext = nc.dram_tensor("hnew", [BS, H], f32, kind="ExternalOutput")
    awout_ext = nc.dram_tensor("awout", [BS, L], f32, kind="ExternalOutput")

    with tile.TileContext(nc) as tc:
        from contextlib import ExitStack
        ctx = ExitStack()
        with ctx:
            const = ctx.enter_context(tc.tile_pool(name="const", bufs=1))
            act = ctx.enter_context(tc.tile_pool(name="act", bufs=1))
            wt = ctx.enter_context(tc.tile_pool(name="wt", bufs=2))
            dram = ctx.enter_context(tc.tile_pool(name="dram", bufs=1,
                                                  space="DRAM"))
            pp = ctx.enter_context(tc.tile_pool(name="pp", bufs=8,
                                                space="PSUM"))

            def psum(name):
                return pp.tile([BS, 512], f32, name=name, tag="ps")

            def psum128(name):
                return pp.tile([128, 512], f32, name=name, tag="ps")

            # ---- embedding gather: data-dependent row DMAs, issued first
            #      (the critical section's entry barrier waits on all prior
            #      work, so it must precede the bulk DMA issuance) ----
            ids_sb = const.tile([1, BS], i32)
            g_engines = [nc.gpsimd, nc.scalar]
            g_etypes = [mybir.EngineType.Pool, mybir.EngineType.Activation]
            g_rows = [16, 16]
            g_tiles = [act.tile([g_rows[e], H], f32, name=f"emb_rows{e}")
                       for e in range(2)]
            g_sems = [nc.alloc_semaphore(f"gather_sem{e}") for e in range(3)]
            nc.gpsimd.dma_start(ids_sb[:], ids_ext[:]).then_inc(g_sems[2], 16)
            with tc.tile_critical():
                for eng in g_engines:
                    eng.wait_ge(g_sems[2], 16)
                for ei in range(2):
                    eng = g_engines[ei]
                    base = sum(g_rows[:ei])
                    for r in range(g_rows[ei]):
                        i = base + r
                        reg = nc.alloc_register(g_etypes[ei], f"embidx{i}")
                        eng.load(reg, ids_sb[0:1, i:i + 1])
                        sv = nc.snap(reg, donate=True, min_val=0,
                                     max_val=V - 1)
                        eng.dma_start(
                            out=g_tiles[ei][r:r + 1, :],
                            in_=emb_ext[bass.ds(sv, 1), :],
                        ).then_inc(g_sems[ei], 16)
                nc.gpsimd.wait_ge(g_sems[0], 16 * g_rows[0])
                nc.scalar.wait_ge(g_sems[1], 16 * g_rows[1])

            # ---- constants / small inputs ----
            ident = const.tile([128, 128], f32)
            nc.sync.dma_start(ident[:], ident_ext[:])
            def bias_bcast(ext, n, name):
                bt = const.tile([128, n], dt, name=name)
                nc.scalar.dma_start(bt[:], ext[0:1, :].partition_broadcast(128))
                return bt

            ab_bc = bias_bcast(ab_ext, L, "ab_bc")
            cb_bc = bias_bcast(cb_ext, H, "cb_bc")
            bih_bc = bias_bcast(bih_ext, 3 * H, "bih_bc")
            bhh_bc = bias_bcast(bhh_ext, 3 * H, "bhh_bc")
            ob_bc = bias_bcast(ob_ext, VS, "ob_bc")

            h0T_sb = const.tile([128, KH * BS], dt)
            nc.sync.dma_start(h0T_sb[:], h0T_ext[:])
            h0_sb = const.tile([BS, H], f32)
            nc.sync.dma_start(h0_sb[:], h0_ext[:])

            ident_bf = const.tile([128, 128], mybir.dt.bfloat16)
            nc.vector.tensor_copy(ident_bf[:], ident[:])
            bd = const.tile([128, BS * BS], dt)
            nc.vector.memset(bd[:], 0.0)

            # warm up ncfw (the collectives firmware): the first collective
            # of a NEFF pays ~20us of cold-start; issue a tiny dummy early
            warm_sb = const.tile([1, 128], f32)
            nc.vector.memset(warm_sb[:], 0.0)
            warm_in = dram.tile([1, 128], f32)
            warm_out = dram.tile([1, 128], f32, addr_space="Shared")
            nc.gpsimd.dma_start(warm_in[:], warm_sb[:])
            nc.gpsimd.collective_compute(
                "AllReduce", mybir.AluOpType.add,
                replica_groups=[list(range(NCORES))],
                ins=[warm_in[:]], outs=[warm_out[:]])

            def pe_transpose(dst_ap, src_ap, p, m, bf=False):
                """dst[m,p] (SBUF, any dtype) = src[p,m].T via PE; p=src parts."""
                if bf:
                    pt = pp.tile([128, 128], mybir.dt.bfloat16, name="ptb",
                                 tag="ps")
                    idn = ident_bf
                else:
                    pt = psum128("pt")
                    idn = ident
                nc.tensor.transpose(pt[:m, :p], src_ap, idn[:p, :p])
                nc.vector.tensor_copy(dst_ap, pt[:m, :p])

            # ---- embT ----
            embT = act.tile([128, KH * BS], dt)
            for k in range(KH):
                for ei in range(2):
                    pe_transpose(
                        embT[:, BS * k + 16 * ei:BS * k + 16 * (ei + 1)],
                        g_tiles[ei][:, 128 * k:128 * (k + 1)], 16, 128)

            # ---- attn_logits = [emb, h0] @ attn_w.T + attn_b ----
            awt = wt.tile([128, 2 * KH * L], dt, name="awt", tag="wattn",
                          bufs=1)
            nc.sync.dma_start(awt[:], awT_ext[:])
            pa = psum("pa")
            for k in range(2 * KH):
                lhsT = (embT[:, BS * k:BS * (k + 1)] if k < KH
                        else h0T_sb[:, BS * (k - KH):BS * (k - KH + 1)])
                nc.tensor.matmul(pa[:, :L], lhsT,
                                 awt[:, L * k:L * (k + 1)],
                                 start=(k == 0), stop=(k == 2 * KH - 1))

            # ---- softmax over L -> aw; awT; block-diag bd ----
            aw_pre = act.tile([BS, L], f32)
            nc.vector.tensor_add(aw_pre[:], pa[:, :L], ab_bc[:BS, :])
            nmx = act.tile([BS, 1], f32)
            nc.vector.tensor_reduce(nmx[:], aw_pre[:], mybir.AxisListType.X,
                                    mybir.AluOpType.max, negate=True)
            aw_exp = act.tile([BS, L], f32)
            se = act.tile([BS, 1], f32)
            nc.scalar.activation(aw_exp[:], aw_pre[:],
                                 mybir.ActivationFunctionType.Exp,
                                 bias=nmx[:], scale=1.0, accum_out=se[:])
            rse = act.tile([BS, 1], f32)
            nc.vector.reciprocal(rse[:], se[:])
            aw_sb = act.tile([BS, L], f32)
            nc.vector.tensor_scalar_mul(aw_sb[:], aw_exp[:], rse[:])
            nc.scalar.dma_start(awout_ext[:], aw_sb[:])

            awT_sb = act.tile([128, BS], dt)
            pe_transpose(awT_sb[:], aw_sb[:], BS, 128)
            # block-diagonal lhsT: bd[:, 32*b + b] = awT[:, b]
            # (bd_z zeroed early, off the critical path)
            nc.vector.tensor_copy(bd[:, 0:BS * BS:BS + 1], awT_sb[:, :])

            # ---- attn_applied[b,:] = aw[b,:] @ enc[b] ----
            patt = [psum("patt0"), psum("patt1")]
            for g in range(BS // EB):
                et = wt.tile([128, EB * H], dt, name="enc", tag="strm", bufs=10)
                nc.sync.dma_start(et[:], enc_ext[:, EB * H * g:
                                                 EB * H * (g + 1)])
                for bb in range(EB):
                    b = EB * g + bb
                    for c in range(2):
                        nc.tensor.matmul(
                            patt[c][:],
                            bd[:, BS * b:BS * (b + 1)],
                            et[:, H * bb + 512 * c:H * bb + 512 * (c + 1)],
                            start=(b == 0), stop=(b == BS - 1),
                            skip_group_check=True)
            att_sb = act.tile([BS, H], dt)
            for c in range(2):
                nc.vector.tensor_copy(att_sb[:, 512 * c:512 * (c + 1)],
                                      patt[c][:])

            # ---- attT ----
            attT = act.tile([128, KH * BS], dt)
            for k in range(KH):
                pe_transpose(attT[:, BS * k:BS * (k + 1)],
                             att_sb[:, 128 * k:128 * (k + 1)], BS, 128,
                             bf=use_bf16)

            # ---- x = relu([emb, att] @ comb_w.T + comb_b) ----
            x_sb = act.tile([BS, H], dt)
            pxs = [psum("px0"), psum("px1")]
            for k in range(2 * KH):
                cwt = wt.tile([128, H], dt, name="cwt", tag="strm", bufs=10)
                nc.sync.dma_start(cwt[:], cwT_ext[128 * k:128 * (k + 1), :])
                lhsT = (embT[:, BS * k:BS * (k + 1)] if k < KH
                        else attT[:, BS * (k - KH):BS * (k - KH + 1)])
                for n in range(2):
                    nc.tensor.matmul(pxs[n][:], lhsT,
                                     cwt[:, 512 * n:512 * (n + 1)],
                                     start=(k == 0), stop=(k == 2 * KH - 1),
                                     skip_group_check=True)
            for n in range(2):
                nc.vector.tensor_add(x_sb[:, 512 * n:512 * (n + 1)],
                                     pxs[n][:],
                                     cb_bc[:BS, 512 * n:512 * (n + 1)])
                nc.vector.tensor_scalar_max(x_sb[:, 512 * n:512 * (n + 1)],
                                            x_sb[:, 512 * n:512 * (n + 1)],
                                            0.0)

            # ---- xT ----
            xT = act.tile([128, KH * BS], dt)
            for k in range(KH):
                pe_transpose(xT[:, BS * k:BS * (k + 1)],
                             x_sb[:, 128 * k:128 * (k + 1)], BS, 128,
                             bf=use_bf16)

            # ---- gx = x @ w_ih.T + b_ih ----
            gx_sb = act.tile([BS, 3 * H], dt)
            pgx = [psum(f"pgx{n}") for n in range(GRU_NC)]
            for k in range(KH):
                for hf in range(2):
                    gwt2 = wt.tile([128, 3 * H // 2], dt, name="gwt2",
                                   tag="strm", bufs=10)
                    nc.sync.dma_start(
                        gwt2[:], wihT_ext[128 * k:128 * (k + 1),
                                          1536 * hf:1536 * (hf + 1)])
                    for nn in range(3):
                        n = 3 * hf + nn
                        nc.tensor.matmul(
                            pgx[n][:], xT[:, BS * k:BS * (k + 1)],
                            gwt2[:, 512 * nn:512 * (nn + 1)],
                            start=(k == 0), stop=(k == KH - 1),
                            skip_group_check=True)
            for n in range(GRU_NC):
                nc.vector.tensor_add(gx_sb[:, 512 * n:512 * (n + 1)],
                                     pgx[n][:],
                                     bih_bc[:BS, 512 * n:512 * (n + 1)])

            # ---- GRU elementwise -> h_new ----
            r_sb = act.tile([BS, H], dt)
            z_sb = act.tile([BS, H], dt)
            n_sb = act.tile([BS, H], dt)
            hnew_sb = act.tile([BS, H], f32)
            hnew_bf = act.tile([BS, H], dt)
            for c in range(2):
                cs = slice(512 * c, 512 * (c + 1))
                t1 = act.tile([BS, 512], dt, tag="gtmp", bufs=4, name="t1")
                nc.vector.tensor_add(t1[:], gx_sb[:, cs], gh_sb[:, cs])
                nc.scalar.activation(r_sb[:, cs], t1[:],
                                     mybir.ActivationFunctionType.Sigmoid)
                t2 = act.tile([BS, 512], dt, tag="gtmp", bufs=4, name="t2")
                nc.vector.tensor_add(t2[:], gx_sb[:, H + 512 * c:H + 512 * (c + 1)],
                                     gh_sb[:, H + 512 * c:H + 512 * (c + 1)])
                nc.scalar.activation(z_sb[:, cs], t2[:],
                                     mybir.ActivationFunctionType.Sigmoid)
                t3 = act.tile([BS, 512], dt, tag="gtmp", bufs=4, name="t3")
                nc.vector.tensor_mul(t3[:], r_sb[:, cs],
                                     gh_sb[:, 2 * H + 512 * c:2 * H + 512 * (c + 1)])
                nc.vector.tensor_add(t3[:], t3[:],
                                     gx_sb[:, 2 * H + 512 * c:2 * H + 512 * (c + 1)])
                nc.scalar.activation(n_sb[:, cs], t3[:],
                                     mybir.ActivationFunctionType.Tanh)
                t4 = act.tile([BS, 512], f32, tag="gtmp2", bufs=2, name="t4")
                nc.vector.tensor_sub(t4[:], h0_sb[:, cs], n_sb[:, cs])
                nc.vector.tensor_mul(t4[:], z_sb[:, cs], t4[:])
                nc.vector.tensor_add(hnew_sb[:, cs], n_sb[:, cs], t4[:])
                nc.vector.tensor_copy(hnew_bf[:, cs], hnew_sb[:, cs])
            nc.scalar.dma_start(hnew_ext[:], hnew_sb[:])

            # ---- AllGather h_new across cores (bf16 payload) ----
            hnew_bounce = dram.tile([BS, H], dt)
            h_all_dram = dram.tile([B, H], dt, addr_space="Shared")
            nc.sync.dma_start(hnew_bounce[:], hnew_bf[:])
            nc.gpsimd.collective_compute(
                "AllGather", mybir.AluOpType.bypass,
                replica_groups=[list(range(NCORES))],
                ins=[hnew_bounce[:]], outs=[h_all_dram[:]])
            h_allT = []
            for m in range(2):
                ha = act.tile([128, H], dt, tag="h_all", name=f"h_all{m}")
                nc.gpsimd.dma_start(ha[:], h_all_dram[128 * m:128 * (m + 1), :])
                for k in range(KH):
                    if m == 0:
                        h_allT.append(act.tile([128, 2 * 128], dt,
                                               name=f"h_allT{k}",
                                               tag=f"h_allT{k}"))
                    pe_transpose(h_allT[k][:, 128 * m:128 * (m + 1)],
                                 ha[:, 128 * k:128 * (k + 1)], 128, 128,
                                 bf=use_bf16)

            # ---- logits = h_all @ out_w.T + out_b; exp partial sums ----
            # vocab in 2 halves of 2000; k-outer so each out_w k-chunk is
            # DMA'd once and serves both batch halves m.
            HNV = NV // 2                  # 4 n-chunks per half
            logits = [act.tile([128, VS], dt, tag=f"logits{m}",
                               name=f"logits{m}") for m in range(2)]
            lsums = act.tile([128, 2 * NV], f32)
            gsumT_sbs = []
            for half in range(2):
                pls = [pp.tile([128, VC], f32, name=f"pl{m}_{n}", tag="ps")
                       for m in range(2) for n in range(HNV)]
                for k in range(KH):
                    owt = wt.tile([128, VS // 2], dt, name="owt", tag="strm",
                                  bufs=10)
                    nc.sync.dma_start(
                        owt[:], owT_ext[128 * k:128 * (k + 1),
                                        (VS // 2) * half:(VS // 2) * (half + 1)])
                    for m in range(2):
                        for n in range(HNV):
                            nc.tensor.matmul(
                                pls[HNV * m + n][:],
                                h_allT[k][:, 128 * m:128 * (m + 1)],
                                owt[:, VC * n:VC * (n + 1)],
                                start=(k == 0), stop=(k == KH - 1),
                                skip_group_check=True)
                for m in range(2):
                    for n in range(HNV):
                        nv = HNV * half + n
                        nc.vector.tensor_add(
                            logits[m][:, VC * nv:VC * (nv + 1)],
                            pls[HNV * m + n][:],
                            ob_bc[:, VC * nv:VC * (nv + 1)])
                        esc = act.tile([128, VC], f32, tag="lpstage",
                                       name="esc", bufs=4)
                        nc.scalar.activation(
                            esc[:], logits[m][:, VC * nv:VC * (nv + 1)],
                            mybir.ActivationFunctionType.Exp,
                            accum_out=lsums[:, NV * m + nv:NV * m + nv + 1])
                # per-half global sum-exp AllReduce; half 0's collective
                # overlaps half 1's matmuls
                lsum_sb = act.tile([128, 2], f32, name=f"lsum{half}",
                                   tag=f"lsum{half}")
                for m in range(2):
                    nc.vector.tensor_reduce(
                        lsum_sb[:, m:m + 1],
                        lsums[:, NV * m + HNV * half:NV * m + HNV * (half + 1)],
                        mybir.AxisListType.X, mybir.AluOpType.add)
                lsumT_sb = act.tile([2, 128], f32, name=f"lsumT{half}",
                                    tag=f"lsumT{half}")
                pe_transpose(lsumT_sb[:], lsum_sb[:], 128, 2)
                lsumT_dram = dram.tile([2, 128], f32, name=f"lsumd{half}")
                gsumT_dram = dram.tile([2, 128], f32, addr_space="Shared",
                                       name=f"gsumd{half}")
                nc.sync.dma_start(lsumT_dram[:], lsumT_sb[:])
                nc.gpsimd.collective_compute(
                    "AllReduce", mybir.AluOpType.add,
                    replica_groups=[list(range(NCORES))],
                    ins=[lsumT_dram[:]], outs=[gsumT_dram[:]])
                gsumT_sb = act.tile([2, 128], f32, name=f"gsumT{half}",
                                    tag=f"gsumT{half}")
                nc.sync.dma_start(gsumT_sb[:], gsumT_dram[:])
                gsumT_sbs.append(gsumT_sb)

            # ---- combine the two per-half partial sums ----
            loggT_sb = act.tile([2, 128], f32)
            nc.vector.tensor_add(loggT_sb[:], gsumT_sbs[0][:], gsumT_sbs[1][:])
            nc.vector.reciprocal(loggT_sb[:], loggT_sb[:])
            nc.scalar.activation(loggT_sb[:], loggT_sb[:],
                                 mybir.ActivationFunctionType.Ln)
            # loggT now holds -log(gsum)
            logg_sb = act.tile([128, 2], f32)
            pe_transpose(logg_sb[:], loggT_sb[:], 2, 128)

            # ---- logp = logits - log(gsum); write out chunked ----
            for m in range(2):
                for n in range(NV):
                    lp = act.tile([128, VC], f32, tag="lpstage", name="lp",
                                  bufs=4)
                    nc.vector.tensor_scalar_sub(
                        lp[:], logits[m][:, VC * n:VC * (n + 1)],
                        logg_sb[:, m:m + 1])
                    (nc.scalar if n % 2 else nc.sync).dma_start(
                        logp_ext[128 * m:128 * (m + 1), VC * n:VC * (n + 1)],
                        lp[:])

    nc.compile()
    return nc


def stage_inputs(input_ids, hidden, encoder_outputs, emb_table,
                 attn_w, attn_b, comb_w, comb_b,
                 w_ih, b_ih, w_hh, b_hh, out_w, out_b, use_bf16=USE_BF16):
    if use_bf16:
        import ml_dtypes
        np_dt = ml_dtypes.bfloat16
    else:
        np_dt = np.float32
    f32 = np.float32

    def cvt(x, dtype):
        return np.ascontiguousarray(np.asarray(x), dtype=dtype)

    ids = np.asarray(input_ids).astype(np.int32).reshape(1, B)
    h0 = cvt(hidden, f32).reshape(B, H)
    enc = cvt(encoder_outputs, np_dt)
    emb = cvt(emb_table, f32)
    # attn_w.T (2H, L) packed into (128, 16*L): [p, L*k + l] = attn_w.T[128k+p, l]
    awT = np.asarray(attn_w).T.astype(np_dt)  # (2H, L)
    awT_packed = np.ascontiguousarray(
        awT.reshape(2 * KH, 128, L).transpose(1, 0, 2).reshape(128, 2 * KH * L))
    ab = cvt(attn_b, np_dt).reshape(1, L)
    cwT = cvt(np.asarray(comb_w).T, np_dt)
    cb = cvt(comb_b, np_dt).reshape(1, H)
    wihT = cvt(np.asarray(w_ih).T, np_dt)
    bih = cvt(b_ih, np_dt).reshape(1, 3 * H)
    whhT = cvt(np.asarray(w_hh).T, np_dt)
    bhh = cvt(b_hh, np_dt).reshape(1, 3 * H)
    owT_full = np.asarray(out_w).T  # (H, V)
    ob = cvt(out_b, np_dt).reshape(1, V)
    ident = np.eye(128, dtype=f32)

    in_maps = []
    for j in range(NCORES):
        bsl = slice(BS * j, BS * (j + 1))
        vsl = slice(VS * j, VS * (j + 1))
        h0_j = np.ascontiguousarray(h0[bsl])
        # packed transposed hidden: h0T[p, BS*k + b] = h0_j[b, 128k + p]
        h0T_j = np.ascontiguousarray(
            h0_j.T.reshape(KH, 128, BS).transpose(1, 0, 2).reshape(128, KH * BS),
            dtype=np_dt)
        # encoder shard packed as (L, BS*H): [l, 1024*b + h]
        enc_j = np.ascontiguousarray(
            enc[bsl].transpose(1, 0, 2).reshape(L, BS * H))
        in_maps.append({
            "ids": np.ascontiguousarray(ids[:, bsl]),
            "emb": emb,
            "h0t": h0T_j,
            "h0": h0_j,
            "enc": enc_j,
            "attn_wt": awT_packed,
            "attn_b": ab,
            "comb_wt": cwT,
            "comb_b": cb,
            "w_iht": wihT,
            "b_ih": bih,
            "w_hht": whhT,
            "b_hh": bhh,
            "out_wt": np.ascontiguousarray(owT_full[:, vsl], dtype=np_dt),
            "out_b": np.ascontiguousarray(ob[:, vsl]),
            "ident": ident,
        })
    return in_maps


def run(inputs, trace=False, trace_cores=None, use_bf16=USE_BF16):
    nc = build_graph(use_bf16)
    in_maps = stage_inputs(**inputs, use_bf16=use_bf16)
    res = run_bass_kernel_spmd(
        nc, in_maps, core_ids=list(range(NCORES)),
        trace=trace, trace_cores=trace_cores)
    r = res.results
    logp = np.concatenate([r[j]["logp"] for j in range(NCORES)],
                          axis=1).astype(np.float32)
    hnew = np.concatenate([r[j]["hnew"] for j in range(NCORES)], axis=0)[None]
    aw = np.concatenate([r[j]["awout"] for j in range(NCORES)], axis=0)
    return (logp, hnew, aw), res


def kernel(**inputs):
    outs, _ = run(inputs, trace=False)
    return outs
